# revision 41
# baseline (speedup 1.0000x reference)
"""Trainium2 Bass kernel for nn_MultiHead (dense transformer layer).

Strategy: pure data-parallel over batch (B=8 -> 8 NeuronCores, no collectives).
Per core: full transformer layer on one [S=1024, D=1024] batch element.

v6 design (on top of the v3 fully-transposed layout):
  - both attention sweeps run behind per-t-pair PE fillers: the c0 sweep
    consumes the remaining QKV projections as fine-grained units (one
    psum group each, just-in-time before the chunk that needs them), and
    the c1 sweep consumes all 32 FF1-c0 m-blocks; the in-order PE queue
    therefore always has ready work at the exp-pipeline stall points.
    The two row-tiled score matmuls of each t-block (PE row groups 0/64)
    are emitted back-to-back into per-head psums so the hardware runs
    them concurrently (the cost model serializes them; hardware does
    not -- worth ~27us there).
  - c-split software pipeline: attention runs queries 0-511 for all 16
    heads first (ACT exp-bound), then LN1-c0, then the c1 attention
    sweep carries all 32 FF1-c0 m-blocks as per-t-pair PE filler inside
    emit_attn -- the exp stream and the FF matmuls share the window, so
    the PE stays ~100% busy from LN1-c0 to the end of the kernel.
  - fp8e4 (e4m3) + DoubleRow perf mode for the Q/K/V projections and the
    ctx accumulation (2x PE rate).  Scores stay bf16 (DoubleRow there
    would need 32-row tiles at base partition 96, which the HW forbids);
    FF1/FF2/proj stay bf16 (fp8 there breaks the 2e-2 gate: measured
    relmax ~1.9e-2 per site in an offline quantization study).
  - fp8 range handling: weights are pre-scaled x32 host-side so w~0.02
    values sit in e4m3's normal range; activations (Q/K/V x32, x true
    scale) stay well under the 240 saturation limit.  The x32 scales
    cancel: QK evac adds 32*bias (Q,K stored as 32*Q, bf16), the exp
    scale absorbs 1/1024, and the softmax denominator ones-column is 32
    so the normalize restores true ctx.
  - ctx DoubleRow pairs t-blocks: the score psum is a 4-bank [P, 2, S]
    tile per t-pair (4 bf16 row-tiled matmuls), one exp per head reads
    [P, 2, 512] N=1024 and writes a [P, 2, 512] fp8 et tile, and the
    ctx DR matmul contracts both t-blocks against the [128, 2, 65]
    V (+32*ones col) stationary into a 1-bank [65, 512] psum.
  - LayerNorm stats / softmax denominators / residuals / FF unchanged
    from v3 (ones-column matmuls, f32r LN inputs, bf16 FF).
"""
from contextlib import ExitStack

import numpy as np

S = 1024
D = 1024
H = 16
DH = 64
DFF = 4096
P = 128
B = 8
NCORES = 8
EPS = 1e-8
WSC = 32.0           # fp8 weight pre-scale
EXPSC = 0.125 / (WSC * WSC)  # exp scale: 1/sqrt(DH) / (32*32)

_RUNNER = None


# ---------------------------------------------------------------- device kernel
def build_nc():
    import concourse.mybir as mybir
    import concourse.tile as tile
    from concourse import bacc

    f32 = mybir.dt.float32
    f32r = mybir.dt.float32r
    bf16 = mybir.dt.bfloat16
    f8 = mybir.dt.float8e4
    AF = mybir.ActivationFunctionType
    ALU = mybir.AluOpType
    DR = mybir.MatmulPerfMode.DoubleRow

    nc = bacc.Bacc("TRN2", target_bir_lowering=False, debug=False)

    # ---- I/O -----------------------------------------------------------------
    xT = nc.declare_dram_parameter("xT", [P, 8, S], bf16, isOutput=False)
    xT8 = nc.declare_dram_parameter("xT8", [P, 8, S], f8, isOutput=False)
    wq = nc.declare_dram_parameter("wq", [8, P, 8, P], f8, isOutput=False)
    wk = nc.declare_dram_parameter("wk", [8, P, 8, P], f8, isOutput=False)
    wv = nc.declare_dram_parameter("wv", [2, P, 8, 512], f8, isOutput=False)
    wp = nc.declare_dram_parameter("wp", [P, 8, D], bf16, isOutput=False)
    wf1 = nc.declare_dram_parameter("wf1", [32, P, 8, P], bf16, isOutput=False)
    wf2 = nc.declare_dram_parameter("wf2", [8, P, 32, P], bf16, isOutput=False)
    qb = nc.declare_dram_parameter("qb", [D], f32, isOutput=False)   # 32x, perm
    kb = nc.declare_dram_parameter("kb", [D], f32, isOutput=False)   # 32x, perm
    vb = nc.declare_dram_parameter("vb", [D], bf16, isOutput=False)  # 32x
    f1b = nc.declare_dram_parameter("f1b", [DFF], f32, isOutput=False)
    f2b = nc.declare_dram_parameter("f2b", [D], f32, isOutput=False)
    pb = nc.declare_dram_parameter("pb", [D], bf16, isOutput=False)
    lng = nc.declare_dram_parameter("lng", [D], f32, isOutput=False)
    lnb = nc.declare_dram_parameter("lnb", [D], f32, isOutput=False)
    fflng = nc.declare_dram_parameter("fflng", [D], f32, isOutput=False)
    fflnb = nc.declare_dram_parameter("fflnb", [D], f32, isOutput=False)
    ones1b = nc.declare_dram_parameter("ones1b", [1, P], bf16, isOutput=False)
    onescol = nc.declare_dram_parameter("onescol", [P, 1], f32r, isOutput=False)
    onespp = nc.declare_dram_parameter("onespp", [P, 1], f32, isOutput=False)
    y = nc.declare_dram_parameter("y", [S, D], f32, isOutput=True)

    def mm(out, lhsT, rhs, start, stop):
        nc.tensor.matmul(out, lhsT, rhs, start=start, stop=stop)

    def mm8(out, lhsT, rhs, start, stop):
        nc.tensor.matmul(out, lhsT, rhs, start=start, stop=stop, perf_mode=DR)

    with tile.TileContext(nc) as tc:
        es_top = ExitStack()

        consts = es_top.enter_context(tc.tile_pool(name="consts", bufs=1))
        mid = es_top.enter_context(tc.tile_pool(name="mid", bufs=1))
        rowp = es_top.enter_context(tc.tile_pool(name="rowp", bufs=1))
        rowbp = es_top.enter_context(tc.tile_pool(name="rowbp", bufs=2))
        scp = es_top.enter_context(tc.tile_pool(name="scp", bufs=2))
        es_qkv = ExitStack()
        xtp = es_qkv.enter_context(tc.tile_pool(name="xtp", bufs=1))
        qkvp = es_qkv.enter_context(tc.tile_pool(name="qkvp", bufs=1))
        es_x8 = ExitStack()
        x8p = es_x8.enter_context(tc.tile_pool(name="x8p", bufs=1))

        # ---- tiles (allocation only; DMA emission is scheduled below) --------
        on1b = consts.tile([1, P], bf16, tag="on1b")
        onc = consts.tile([P, 1], f32r, tag="onc")
        onpp = consts.tile([P, 1], f32, tag="onpp")
        eps_t = consts.tile([1, 1], f32, tag="eps")
        qb_sb = consts.tile([P, 8], f32, tag="qb")
        kb_sb = consts.tile([P, 8], f32, tag="kb")
        f1b_sb = consts.tile([P, 32], f32, tag="f1b")
        f2b_sb = consts.tile([P, 8], f32, tag="f2b")
        gb1 = consts.tile([P, 8], f32, tag="gb1")
        bb1 = consts.tile([P, 8], f32, tag="bb1")
        gb2 = consts.tile([P, 8], f32, tag="gb2")
        bb2 = consts.tile([P, 8], f32, tag="bb2")
        vb_row = consts.tile([1, D], bf16, tag="vbrow")
        pb_row = consts.tile([1, D], bf16, tag="pbrow")

        CT = mid.tile([P, 8, S], f32r, tag="ctff")     # ctx+resid, later FFT
        SQ = rowp.tile([P, 8, 512], bf16, tag="sq")    # squares (per c-half)
        O1T = mid.tile([P, 8, S], bf16, tag="o1t")
        XT = xtp.tile([P, 8, S], bf16, tag="xt")
        XT8 = x8p.tile([P, 8, S], f8, tag="xt8")
        QT = qkvp.tile([P, 8, S], bf16, tag="qt")
        KT = qkvp.tile([P, 8, S], bf16, tag="kt")
        Vp = qkvp.tile([P, 8, H * (DH + 1)], f8, tag="vp")
        Vp5 = Vp[:].rearrange("p i (hh e) -> p i hh e", e=DH + 1)

        # critical-path DMAs first: the first stationary weight tile, then XT
        xTr = xT[:].rearrange("(ko p) s -> p ko s", p=P)
        xT8r = xT8[:].rearrange("(ko p) s -> p ko s", p=P)

        es_ph1 = ExitStack()
        w1p = es_ph1.enter_context(tc.tile_pool(name="w1p", bufs=3))
        wj0q = w1p.tile([P, 8, P], f8, tag="wqk", name="wj0q")
        nc.sync.dma_start(wj0q[:], wq[0])
        for k in range(8):
            nc.sync.dma_start(XT8[:, k, :], xT8r[:, k, :])
        wj0k = w1p.tile([P, 8, P], f8, tag="wqk", name="wj0k")
        nc.sync.dma_start(wj0k[:], wk[0])
        wj1q = w1p.tile([P, 8, P], f8, tag="wqk", name="wj1q")
        nc.sync.dma_start(wj1q[:], wq[1])
        nc.sync.dma_start(qb_sb[:], qb[:].rearrange("(j p) -> p j", p=P))
        nc.sync.dma_start(kb_sb[:], kb[:].rearrange("(j p) -> p j", p=P))
        wvp = es_ph1.enter_context(tc.tile_pool(name="wvp", bufs=1))
        etp = es_ph1.enter_context(tc.tile_pool(name="etp", bufs=2))
        drp = es_ph1.enter_context(tc.tile_pool(name="drp", bufs=1))
        ps_sp = es_ph1.enter_context(
            tc.tile_pool(name="ps_sp", bufs=2, space="PSUM"))
        ps_cp = es_ph1.enter_context(
            tc.tile_pool(name="ps_cp", bufs=2, space="PSUM"))
        es_qkps = ExitStack()
        ps_qk = es_qkps.enter_context(
            tc.tile_pool(name="ps_qk", bufs=1, space="PSUM"))

        def emit_qk(j, pre=None):
            """Q and K projections for feature block j (fp8 DoubleRow over
            k-pairs).  psum pair tile: c0 -> bank 0, c1 -> bank 1, one fused
            relu evac writing 32*Q (resp 32*K) as fp8."""
            for wi, (wdram, bias_sb, out) in enumerate(
                    ((wq, qb_sb, QT), (wk, kb_sb, KT))):
                if pre is not None and pre[wi] is not None:
                    wj = pre[wi]
                else:
                    wj = w1p.tile([P, 8, P], f8, tag="wqk")
                    nc.sync.dma_start(wj[:], wdram[j])
                pt = ps_qk.tile([P, S], f32, tag="pqk")
                for c in range(2):
                    for t in range(4):
                        mm8(pt[:, c * 512:(c + 1) * 512],
                            wj[:, 2 * t:2 * t + 2, :],
                            XT8[:, 2 * t:2 * t + 2, c * 512:(c + 1) * 512],
                            start=(t == 0), stop=(t == 3))
                # relu(x+32b) on DVE: keeps phase-A ACT nearly exp-only
                nc.vector.tensor_scalar(out[:, j, :], pt[:],
                                        bias_sb[:, j:j + 1], 0.0,
                                        ALU.add, ALU.max)

        def emit_v(c):
            """V projection for dout half c (heads 8c..8c+7), fp8 DoubleRow."""
            wvc = wvp.tile([P, 8, 512], f8, tag="wvc")
            for k in range(8):
                nc.sync.dma_start(wvc[:, k, :], wv[:, k, c * 512:(c + 1) * 512])
            for i2 in range(4):
                pv = ps_qk.tile([P, S], f32, tag="pqk")
                for io in range(2):
                    i = 2 * i2 + io
                    hv = slice(io * 512, (io + 1) * 512)
                    for t in range(4):
                        mm8(pv[:, hv],
                            XT8[:, 2 * t:2 * t + 2, i * 128:(i + 1) * 128],
                            wvc[:, 2 * t:2 * t + 2, :],
                            start=(t == 0), stop=False)
                    mm(pv[:, hv], on1b[:], vb_row[:, c * 512:(c + 1) * 512],
                       start=False, stop=True)
                pv4 = pv[:].rearrange("p (io hh e) -> p io hh e", io=2, e=DH)
                nc.scalar.activation(
                    Vp5[:, 2 * i2:2 * i2 + 2, c * 8:(c + 1) * 8, 0:DH],
                    pv4[:], AF.Relu)

        def emit_attn(j, cset=(0, 1), cp_pool=None, filler=None):
            """Attention for head pair (2j, 2j+1).

            scores: bf16 row-tiled (u pairs at bases 0/64) into a 2-bank
            [P, 2, 512] psum per (t-pair, head); one exp per head reads
            [P, 2, 512] N=1024 and writes an fp8 et tile; ctx:
            [128,2,65]x[128,2,512] fp8 DoubleRow over t-pairs into a
            1-bank [65,512] psum.  `filler()` (if given) is called once
            per t-pair to splice independent PE work (FF1 blocks) into
            the queue so the exp stream never starves the PE."""
            cpp = cp_pool or ps_cp
            for c in cset:
                cs = slice(c * 512, (c + 1) * 512)
                cps = [cpp.tile([65, 512], f32, tag="cp",
                                name=f"cp_{j}_{c}_{u}") for u in range(2)]
                for tp in range(4):
                    # two 2-bank psums per t-pair (one per head) allocated
                    # up front, writes interleaved: the row-tiled score
                    # pair (bases 0/64) is adjacent in the PE queue so HW
                    # overlaps it, and the staggered exp completions match
                    # the staggered slot-reuse order of the next t-pair
                    sps = [ps_sp.tile([P, 2, 512], f32, tag="sp",
                                      name=f"sp{j}_{c}_{tp}_{u}")
                           for u in range(2)]
                    for i in range(2):
                        t = 2 * tp + i
                        for u in range(2):
                            r0 = 64 * u
                            mm(sps[u][:, i, :],
                               KT[r0:r0 + 64, j, t * 128:(t + 1) * 128],
                               QT[r0:r0 + 64, j, cs], start=True, stop=True)
                    for u in range(2):
                        et = etp.tile([P, 2, 512], f8, tag="et")
                        nc.scalar.activation(et[:], sps[u][:], AF.Exp,
                                             scale=EXPSC)
                        mm8(cps[u][:], Vp5[:, 2 * tp:2 * tp + 2, 2 * j + u, :],
                            et[:], start=(tp == 0), stop=(tp == 3))
                    if filler is not None:
                        filler()
                # normalize by denominator row + write CT (true scale: the
                # x32 of V cancels against the 32-valued ones column)
                for u in range(2):
                    dr = drp.tile([1, 512], f32, tag="dr")
                    nc.vector.reciprocal(dr[:], cps[u][64:65, :])
                    db = drp.tile([64, 512], f32, tag="db")
                    nc.gpsimd.partition_broadcast(db[:], dr[:], channels=64)
                    r0 = 64 * u
                    nc.vector.tensor_tensor(CT[r0:r0 + 64, j, cs],
                                            cps[u][0:64, :], db[:], ALU.mult)

        # ------- LayerNorm building blocks (transposed layout) ----------------
        def emit_resid(dst, other, j, cs, eng=None):
            (eng or nc.vector).tensor_tensor(dst[:, j, cs], dst[:, j, cs],
                                             other[:, j, cs], ALU.add)

        def emit_sq(c, src, js, eng):
            cs = slice(c * 512, (c + 1) * 512)
            for j in js:
                eng.tensor_tensor(SQ[:, j, :], src[:, j, cs], src[:, j, cs],
                                  ALU.mult)

        def emit_stats(ln_ps, c, src, nm):
            cs = slice(c * 512, (c + 1) * 512)
            psS = ln_ps.tile([1, 512], f32, tag="sums", name=f"psS_{nm}_{c}")
            psQ = ln_ps.tile([1, 512], f32, tag="sumq", name=f"psQ_{nm}_{c}")
            for j in range(8):
                mm(psS[:], onc[:], src[:, j, cs], start=(j == 0), stop=(j == 7))
                mm(psQ[:], onc[:], SQ[:, j, :], start=(j == 0), stop=(j == 7))
            return psS, psQ

        def emit_finalize(psS, psQ):
            """mean/var -> alpha (=1/std) and r2 (=mu/std), broadcast rows."""
            mu = rowp.tile([1, 512], f32, tag="mu")
            nc.scalar.activation(mu[:], psS[:], AF.Copy, scale=1.0 / D)
            ex2 = rowp.tile([1, 512], f32, tag="ex2")
            nc.scalar.activation(ex2[:], psQ[:], AF.Copy, scale=1.0 / D)
            var = rowp.tile([1, 512], f32, tag="var")
            nc.vector.tensor_tensor(var[:], mu[:], mu[:], ALU.mult)
            nc.vector.tensor_tensor(var[:], ex2[:], var[:], ALU.subtract)
            al = rowp.tile([1, 512], f32, tag="al")
            nc.scalar.activation(al[:], var[:], AF.Sqrt, bias=eps_t[:])
            nc.vector.reciprocal(al[:], al[:])
            r2 = rowp.tile([1, 512], f32, tag="r2")
            nc.vector.tensor_tensor(r2[:], mu[:], al[:], ALU.mult)
            ab = rowbp.tile([P, 512], f32, tag="ab")
            nc.gpsimd.partition_broadcast(ab[:], al[:], channels=P)
            rb = rowbp.tile([P, 512], f32, tag="rb")
            nc.gpsimd.partition_broadcast(rb[:], r2[:], channels=P)
            return ab, rb

        def emit_apply(c, src, gcol, bcol, out, ab, rb, dve_js, js=tuple(range(8))):
            """out = (src*alpha - r2)*g + b; split across DVE and Pool."""
            cs = slice(c * 512, (c + 1) * 512)
            for j in js:
                if j in dve_js:
                    sc = scp.tile([P, 512], f32, tag="scv")
                    nc.vector.tensor_tensor(sc[:], src[:, j, cs], ab[:],
                                            ALU.mult)
                    nc.vector.tensor_tensor(sc[:], sc[:], rb[:], ALU.subtract)
                    nc.vector.tensor_scalar(out[:, j, cs], sc[:],
                                            gcol[:, j:j + 1], bcol[:, j:j + 1],
                                            ALU.mult, ALU.add)
                else:
                    sc = scp.tile([P, 512], f32, tag="scp")
                    nc.gpsimd.tensor_tensor(sc[:], src[:, j, cs], ab[:],
                                            ALU.mult)
                    nc.gpsimd.tensor_tensor(sc[:], sc[:], rb[:], ALU.subtract)
                    nc.gpsimd.tensor_scalar(out[:, j, cs], sc[:],
                                            gcol[:, j:j + 1], bcol[:, j:j + 1],
                                            ALU.mult, ALU.add)

        DVE_JS = (0, 1, 2)   # Pool is faster per op; give it the bigger share

        # ---- phase A: QKV + attention, interleaved ---------------------------
        emit_qk(0, pre=(wj0q, wj0k))
        # small consts stream in behind the first weight loads
        nc.sync.dma_start(onpp[:], onespp[:])
        nc.sync.dma_start(on1b[:], ones1b[:])
        nc.sync.dma_start(vb_row[:], vb[None, :])
        # softmax-denominator ones column, value 32 (cancels V's x32)
        vp_col = Vp[:].rearrange("p i (hh e) -> p (i hh) e", e=DH + 1)[:, :, DH]
        nc.scalar.activation(vp_col, onpp[:].to_broadcast((P, 8 * H)), AF.Copy,
                             scale=WSC)
        emit_qk(1, pre=(wj1q, None))
        nc.sync.dma_start(onc[:], onescol[:])
        nc.vector.memset(eps_t[:], EPS)
        nc.sync.dma_start(gb1[:], lng[:].rearrange("(j p) -> p j", p=P))
        nc.sync.dma_start(bb1[:], lnb[:].rearrange("(j p) -> p j", p=P))
        nc.sync.dma_start(gb2[:], fflng[:].rearrange("(j p) -> p j", p=P))
        nc.sync.dma_start(bb2[:], fflnb[:].rearrange("(j p) -> p j", p=P))
        nc.sync.dma_start(f1b_sb[:], f1b[:].rearrange("(j p) -> p j", p=P))
        nc.sync.dma_start(f2b_sb[:], f2b[:].rearrange("(j p) -> p j", p=P))
        nc.sync.dma_start(pb_row[:], pb[None, :])
        # bf16 XT (residual path only) streams behind the fp8 critical path
        for k in range(8):
            nc.sync.dma_start(XT[:, k, :], xTr[:, k, :])
        emit_v(0)
        emit_qk(2)
        # ---- c0 sweep: attention on queries 0..511 for all pairs, QKV
        # projections for later blocks interleaved behind the exp stream.
        emit_attn(0, (0,))
        emit_qk(3)
        emit_resid(CT, XT, 0, slice(0, 512))
        emit_sq(0, CT, (0,), nc.gpsimd)
        emit_v(1)
        emit_attn(1, (0,))
        emit_qk(4)
        emit_resid(CT, XT, 1, slice(0, 512))
        emit_sq(0, CT, (1,), nc.gpsimd)
        emit_attn(2, (0,))
        emit_qk(5)
        emit_resid(CT, XT, 2, slice(0, 512))
        emit_sq(0, CT, (2,), nc.gpsimd)
        emit_attn(3, (0,))
        emit_qk(6)
        emit_resid(CT, XT, 3, slice(0, 512))
        emit_sq(0, CT, (3,), nc.gpsimd)
        emit_attn(4, (0,))
        emit_qk(7)
        emit_resid(CT, XT, 4, slice(0, 512))
        emit_sq(0, CT, (4,), nc.gpsimd)
        emit_attn(5, (0,))
        emit_resid(CT, XT, 5, slice(0, 512))
        emit_sq(0, CT, (5,), nc.gpsimd)
        emit_attn(6, (0,))
        emit_resid(CT, XT, 6, slice(0, 512))
        emit_sq(0, CT, (6,), nc.gpsimd)
        emit_attn(7, (0,))
        emit_resid(CT, XT, 7, slice(0, 512))
        emit_sq(0, CT, (7,), nc.gpsimd)
        # QK/V psum no longer needed; swap those banks to the LN1-c0 stats
        es_qkps.close()
        es_lnA = ExitStack()
        ln_psA = es_lnA.enter_context(
            tc.tile_pool(name="ln_psA", bufs=1, space="PSUM"))
        psS0, psQ0 = emit_stats(ln_psA, 0, CT, "ln1")
        ab0, rb0 = emit_finalize(psS0, psQ0)
        es_lnA.close()
        # apply-c0 split DVE/Pool: the first FF1 filler block gates on it
        emit_apply(0, CT, gb1, bb1, O1T, ab0, rb0, (0, 1, 2))

        # ---- overlap window: attention c1 (exp-bound on ACT) carries the
        # FF1-c0 matmuls as PE filler, one m-block per t-pair.
        es_x8.close()   # free XT8 (projections done)
        es_ffa = ExitStack()
        ffap = es_ffa.enter_context(tc.tile_pool(name="ffap", bufs=1))
        wf1p = es_ffa.enter_context(tc.tile_pool(name="wf1p", bufs=3))
        ff_ps = es_ffa.enter_context(
            tc.tile_pool(name="ff_ps", bufs=2, space="PSUM"))
        H1 = ffap.tile([P, 32, 512], bf16, tag="h1")
        FFT = mid.tile([P, 8, S], f32r, tag="ctff")  # reuse CT buffer

        def emit_ff1_block(m, c, relu_on_act):
            cs = slice(c * 512, (c + 1) * 512)
            wm = wf1p.tile([P, 8, P], bf16, tag="wf1")
            nc.sync.dma_start(wm[:], wf1[m])
            pt = ff_ps.tile([P, 512], f32, tag="ff")
            for k in range(8):
                mm(pt[:], wm[:, k, :], O1T[:, k, cs],
                   start=(k == 0), stop=(k == 7))
            if relu_on_act:
                nc.scalar.activation(H1[:, m, :], pt[:], AF.Relu,
                                     bias=f1b_sb[:, m:m + 1])
            else:
                # DVE relu evac: keeps the overlap window's ACT exp-only
                nc.vector.tensor_scalar(H1[:, m, :], pt[:],
                                        f1b_sb[:, m:m + 1], 0.0,
                                        ALU.add, ALU.max)

        ff1_m = iter(range(32))

        def ff1_filler():
            m = next(ff1_m, None)
            if m is not None:
                emit_ff1_block(m, 0, relu_on_act=False)

        for j in range(8):
            emit_attn(j, (1,), filler=ff1_filler)
            emit_resid(CT, XT, j, slice(512, 1024))
            emit_sq(1, CT, (j,), nc.gpsimd)
        for m in ff1_m:  # any filler slots the attention loop didn't consume
            emit_ff1_block(m, 0, relu_on_act=False)

        es_ph1.close()
        es_qkv.close()   # free XT / XT8 / QT / KT / Vp

        es_ph2 = ExitStack()
        ln_ps1 = es_ph2.enter_context(
            tc.tile_pool(name="ln_ps1", bufs=1, space="PSUM"))

        # LN1-c1 chain; the FF2-c0 matmuls right after keep the PE busy
        # while finalize/apply run on ACT/DVE/Pool.
        psS1, psQ1 = emit_stats(ln_ps1, 1, CT, "ln1")
        ab1, rb1 = emit_finalize(psS1, psQ1)
        emit_apply(1, CT, gb1, bb1, O1T, ab1, rb1, (0, 1, 2))

        # ---- phase C pools (FF + LN2 + proj) ---------------------------------
        es_ph3 = ExitStack()
        ffp = es_ph3.enter_context(tc.tile_pool(name="ffp", bufs=1))
        wf2p = es_ph3.enter_context(tc.tile_pool(name="wf2p", bufs=2))
        ytp = es_ph3.enter_context(tc.tile_pool(name="ytp", bufs=3))
        pj_ps = es_ph3.enter_context(
            tc.tile_pool(name="pj_ps", bufs=2, space="PSUM"))

        O2T = ffp.tile([P, 8, S], bf16, tag="o2t")
        WP = ffp.tile([P, 8, D], bf16, tag="wp")

        def emit_ff1(c):
            for m in range(32):
                emit_ff1_block(m, c, relu_on_act=True)

        def emit_ff2(c, pre=()):
            cs = slice(c * 512, (c + 1) * 512)
            for j in range(8):
                if j < len(pre):
                    w2j = pre[j]
                else:
                    w2j = wf2p.tile([P, 32, P], bf16, tag="w2j")
                    nc.sync.dma_start(w2j[:], wf2[j])
                pt = ff_ps.tile([P, 512], f32, tag="ff")
                for m in range(32):
                    mm(pt[:], w2j[:, m, :], H1[:, m, :],
                       start=(m == 0), stop=(m == 31))
                # fused evac: FFT = (psum + f2b) + O1T  (bias + residual)
                nc.vector.scalar_tensor_tensor(
                    FFT[:, j, cs], pt[:], f2b_sb[:, j:j + 1],
                    O1T[:, j, cs], ALU.add, ALU.add)

        def emit_proj(iset, split_last=False):
            for i in iset:
                yt = ytp.tile([P, D], f32, tag="yt")
                pp = pj_ps.tile([P, D], f32, tag="pj")
                split = split_last and i == iset[-1]
                for dh in range(2):
                    ds_ = slice(dh * 512, (dh + 1) * 512)
                    for k in range(8):
                        mm(pp[:, ds_], O2T[:, k, i * 128:(i + 1) * 128],
                           WP[:, k, ds_], start=(k == 0), stop=False)
                    mm(pp[:, ds_], on1b[:], pb_row[:, ds_],
                       start=False, stop=True)
                    if split:
                        nc.scalar.activation(yt[:, ds_], pp[:, ds_], AF.Copy)
                        nc.sync.dma_start(y[i * 128:(i + 1) * 128, ds_],
                                          yt[:, ds_])
                if not split:
                    nc.scalar.activation(yt[:], pp[:], AF.Copy)
                    nc.sync.dma_start(y[i * 128:(i + 1) * 128, :], yt[:])

        for k in range(8):
            nc.sync.dma_start(WP[:, k, :], wp[:, k, :])
        emit_ff2(0)
        emit_ff1(1)
        # LN2 c0: chain overlaps FF1 c1 matmuls (residual fused into FF2 evac)
        emit_sq(0, FFT, tuple(range(8)), nc.gpsimd)
        psS2, psQ2 = emit_stats(ln_ps1, 0, FFT, "ln2")
        ab2, rb2 = emit_finalize(psS2, psQ2)
        emit_apply(0, FFT, gb2, bb2, O2T, ab2, rb2, (0, 1))
        emit_ff2(1)
        # LN2 c1 chain overlaps proj i0-i1 (residual fused into FF2 evac)
        emit_sq(1, FFT, tuple(range(8)), nc.gpsimd)
        emit_proj((0, 1))
        psS3, psQ3 = emit_stats(ln_ps1, 1, FFT, "ln2")
        ab3, rb3 = emit_finalize(psS3, psQ3)
        emit_proj((2, 3))
        emit_apply(1, FFT, gb2, bb2, O2T, ab3, rb3, DVE_JS)
        emit_proj((4, 5))
        emit_proj((6,), split_last=True)
        emit_proj((7,), split_last=True)

        es_ph3.close()
        es_ffa.close()
        es_ph2.close()
        es_top.close()

    nc.compile()
    return nc


# ---------------------------------------------------------------- host wrapper
class _SpmdRunner:
    """Compile once, run repeatedly (mirrors bass2jax.run_bass_via_pjrt)."""

    def __init__(self, nc, n_cores):
        import jax
        from jax.sharding import Mesh, PartitionSpec
        from jax.experimental.shard_map import shard_map
        import concourse.mybir as mybir
        from concourse import bass2jax
        from concourse.bass2jax import _bass_exec_p, install_neuronx_cc_hook

        install_neuronx_cc_hook()
        self.n_cores = n_cores
        partition_name = (
            nc.partition_id_tensor.name if nc.partition_id_tensor else None
        )
        in_names, out_names, out_avals, zero_outs = [], [], [], []
        for alloc in nc.m.functions[0].allocations:
            if not isinstance(alloc, mybir.MemoryLocationSet):
                continue
            name = alloc.memorylocations[0].name
            if alloc.kind == "ExternalInput":
                if name != partition_name:
                    in_names.append(name)
            elif alloc.kind == "ExternalOutput":
                shape = tuple(alloc.tensor_shape)
                dtype = mybir.dt.np(alloc.dtype)
                out_names.append(name)
                out_avals.append(jax.core.ShapedArray(shape, dtype))
                zero_outs.append(np.zeros(shape, dtype))
        self.in_names = in_names
        self.out_names = out_names
        self.out_avals = out_avals
        self.zero_outs = zero_outs
        n_params = len(in_names)
        n_outs = len(out_avals)
        all_in_names = in_names + out_names
        if partition_name is not None:
            all_in_names.append(partition_name)
        donate = tuple(range(n_params, n_params + n_outs))

        def _body(*args):
            operands = list(args)
            if partition_name is not None:
                operands.append(bass2jax.partition_id_tensor())
            outs = _bass_exec_p.bind(
                *operands,
                out_avals=tuple(out_avals),
                in_names=tuple(all_in_names),
                out_names=tuple(out_names),
                lowering_input_output_aliases=(),
                sim_require_finite=True,
                sim_require_nnan=True,
                nc=nc,
            )
            return tuple(outs)

        import jax as _jax
        devices = _jax.devices()[:n_cores]
        assert len(devices) == n_cores
        mesh = Mesh(np.asarray(devices), ("core",))
        in_specs = (PartitionSpec("core"),) * (n_params + n_outs)
        out_specs = (PartitionSpec("core"),) * n_outs
        self.fn = _jax.jit(
            shard_map(_body, mesh=mesh, in_specs=in_specs,
                      out_specs=out_specs, check_rep=False),
            donate_argnums=donate,
            keep_unused=True,
        )

    def prep_inputs(self, in_maps):
        per_core = [[np.asarray(m[n]) for n in self.in_names] for m in in_maps]
        return [
            np.concatenate([per_core[c][i] for c in range(self.n_cores)], axis=0)
            for i in range(len(self.in_names))
        ]

    def zeros(self):
        return [
            np.zeros((self.n_cores * z.shape[0], *z.shape[1:]), z.dtype)
            for z in self.zero_outs
        ]

    def run_device(self, concat_in):
        return self.fn(*concat_in, *self.zeros())

    def split(self, out_arrs):
        return [
            {
                name: np.asarray(out_arrs[i]).reshape(
                    self.n_cores, *self.out_avals[i].shape)[c]
                for i, name in enumerate(self.out_names)
            }
            for c in range(self.n_cores)
        ]


def make_in_maps(**inputs):
    import ml_dtypes
    BF16 = np.dtype(ml_dtypes.bfloat16)
    F8 = np.dtype(ml_dtypes.float8_e4m3)
    f32 = np.float32
    q = np.ascontiguousarray(np.asarray(inputs["queries"], dtype=f32))

    def arr(name):
        return np.ascontiguousarray(np.asarray(inputs[name], dtype=f32))

    Qw, Kw, Vw = arr("Qw"), arr("Kw"), arr("Vw")
    proj_w, ff1_w, ff2_w = arr("proj_w"), arr("ff1_w"), arr("ff2_w")

    # packed weight layouts (all-contiguous device DMAs)
    def pack_lhsT(w, nj):  # [dout, din] -> [j, p(k), ko, mc]
        return np.ascontiguousarray(
            w.reshape(nj, P, 8, P).transpose(0, 3, 2, 1))

    def pack_rhs(w):  # [dout, din] -> W^T as [p(k), ko, dout]
        return np.ascontiguousarray(
            w.T.reshape(8, P, w.shape[0]).transpose(1, 0, 2))

    shared = {
        "wq": pack_lhsT(Qw * WSC, 8).astype(F8),
        "wk": pack_lhsT(Kw * WSC, 8).astype(F8),
        "wv": np.ascontiguousarray(
            pack_rhs(Vw * WSC).reshape(P, 8, 2, 512).transpose(2, 0, 1, 3)
        ).astype(F8),
        "wp": pack_rhs(proj_w).astype(BF16),
        "wf1": pack_lhsT(ff1_w, 32).astype(BF16),
        "wf2": np.ascontiguousarray(
            ff2_w.T.reshape(32, P, 8, P).transpose(2, 1, 0, 3)).astype(BF16),
        "qb": arr("Qb") * WSC, "kb": arr("Kb") * WSC,
        "vb": (arr("Vb") * WSC).astype(BF16),
        "f1b": arr("ff1_b"), "f2b": arr("ff2_b"),
        "pb": arr("proj_b").astype(BF16),
        "lng": arr("ln_g"), "lnb": arr("ln_b"),
        "fflng": arr("ffln_g"), "fflnb": arr("ffln_b"),
        "ones1b": np.ones((1, P), dtype=f32).astype(BF16),
        "onescol": np.ones((P, 1), dtype=f32),
        "onespp": np.ones((P, 1), dtype=f32),
    }
    in_maps = []
    for b in range(B):
        m = dict(shared)
        xb = np.ascontiguousarray(
            q[b].T.reshape(8, P, S).transpose(1, 0, 2))
        m["xT"] = xb.astype(BF16)
        m["xT8"] = xb.astype(F8)
        in_maps.append(m)
    return in_maps


def get_runner():
    global _RUNNER
    if _RUNNER is None:
        nc = build_nc()
        _RUNNER = _SpmdRunner(nc, NCORES)
    return _RUNNER


def kernel(**inputs):
    runner = get_runner()
    in_maps = make_in_maps(**inputs)
    res = runner.split(runner.run_device(runner.prep_inputs(in_maps)))
    out = np.stack([res[c]["y"] for c in range(NCORES)], axis=0)
    return out.astype(np.float32)


# revision 42
# speedup vs baseline: 1.2810x; 1.2810x over previous
"""Trainium2 Bass kernel for nn_MultiHead (dense transformer layer).

Strategy: pure data-parallel over batch (B=8 -> 8 NeuronCores, no collectives).
Per core: full transformer layer on one [S=1024, D=1024] batch element.

v6 design (on top of the v3 fully-transposed layout):
  - both attention sweeps run behind per-t-pair PE fillers: the c0 sweep
    consumes the remaining QKV projections as fine-grained units (one
    psum group each, just-in-time before the chunk that needs them), and
    the c1 sweep consumes all 32 FF1-c0 m-blocks; the in-order PE queue
    therefore always has ready work at the exp-pipeline stall points.
    The two row-tiled score matmuls of each t-block (PE row groups 0/64)
    are emitted back-to-back into per-head psums so the hardware runs
    them concurrently (the cost model serializes them; hardware does
    not -- worth ~27us there).
  - c-split software pipeline: attention runs queries 0-511 for all 16
    heads first (ACT exp-bound), then LN1-c0, then the c1 attention
    sweep carries all 32 FF1-c0 m-blocks as per-t-pair PE filler inside
    emit_attn -- the exp stream and the FF matmuls share the window, so
    the PE stays ~100% busy from LN1-c0 to the end of the kernel.
  - fp8e4 (e4m3) + DoubleRow perf mode for the Q/K/V projections and the
    ctx accumulation (2x PE rate).  Scores stay bf16 (DoubleRow there
    would need 32-row tiles at base partition 96, which the HW forbids);
    FF1/FF2/proj stay bf16 (fp8 there breaks the 2e-2 gate: measured
    relmax ~1.9e-2 per site in an offline quantization study).
  - fp8 range handling: weights are pre-scaled x32 host-side so w~0.02
    values sit in e4m3's normal range; activations (Q/K/V x32, x true
    scale) stay well under the 240 saturation limit.  The x32 scales
    cancel: QK evac adds 32*bias (Q,K stored as 32*Q, bf16), the exp
    scale absorbs 1/1024, and the softmax denominator ones-column is 32
    so the normalize restores true ctx.
  - ctx DoubleRow pairs t-blocks: the score psum is a 4-bank [P, 2, S]
    tile per t-pair (4 bf16 row-tiled matmuls), one exp per head reads
    [P, 2, 512] N=1024 and writes a [P, 2, 512] fp8 et tile, and the
    ctx DR matmul contracts both t-blocks against the [128, 2, 65]
    V (+32*ones col) stationary into a 1-bank [65, 512] psum.
  - LayerNorm stats / softmax denominators / residuals / FF unchanged
    from v3 (ones-column matmuls, f32r LN inputs, bf16 FF).
"""
from contextlib import ExitStack

import numpy as np

S = 1024
D = 1024
H = 16
DH = 64
DFF = 4096
P = 128
B = 8
NCORES = 8
EPS = 1e-8
WSC = 32.0           # fp8 weight pre-scale
EXPSC = 0.125 / (WSC * WSC)  # exp scale: 1/sqrt(DH) / (32*32)

_RUNNER = None


# ---------------------------------------------------------------- device kernel
def build_nc():
    import concourse.mybir as mybir
    import concourse.tile as tile
    from concourse import bacc

    f32 = mybir.dt.float32
    f32r = mybir.dt.float32r
    bf16 = mybir.dt.bfloat16
    f8 = mybir.dt.float8e4
    AF = mybir.ActivationFunctionType
    ALU = mybir.AluOpType
    DR = mybir.MatmulPerfMode.DoubleRow

    nc = bacc.Bacc("TRN2", target_bir_lowering=False, debug=False)

    # ---- I/O -----------------------------------------------------------------
    xT = nc.declare_dram_parameter("xT", [P, 8, S], bf16, isOutput=False)
    xT8 = nc.declare_dram_parameter("xT8", [P, 8, S], f8, isOutput=False)
    wq = nc.declare_dram_parameter("wq", [8, P, 8, P], f8, isOutput=False)
    wk = nc.declare_dram_parameter("wk", [8, P, 8, P], f8, isOutput=False)
    wv = nc.declare_dram_parameter("wv", [2, P, 8, 512], f8, isOutput=False)
    wp = nc.declare_dram_parameter("wp", [P, 8, D], bf16, isOutput=False)
    wf1 = nc.declare_dram_parameter("wf1", [32, P, 8, P], bf16, isOutput=False)
    wf2 = nc.declare_dram_parameter("wf2", [8, P, 32, P], bf16, isOutput=False)
    qb = nc.declare_dram_parameter("qb", [D], f32, isOutput=False)   # 32x, perm
    kb = nc.declare_dram_parameter("kb", [D], f32, isOutput=False)   # 32x, perm
    vb = nc.declare_dram_parameter("vb", [D], bf16, isOutput=False)  # 32x
    f1b = nc.declare_dram_parameter("f1b", [DFF], f32, isOutput=False)
    f2b = nc.declare_dram_parameter("f2b", [D], f32, isOutput=False)
    pb = nc.declare_dram_parameter("pb", [D], bf16, isOutput=False)
    lng = nc.declare_dram_parameter("lng", [D], f32, isOutput=False)
    lnb = nc.declare_dram_parameter("lnb", [D], f32, isOutput=False)
    fflng = nc.declare_dram_parameter("fflng", [D], f32, isOutput=False)
    fflnb = nc.declare_dram_parameter("fflnb", [D], f32, isOutput=False)
    ones1b = nc.declare_dram_parameter("ones1b", [1, P], bf16, isOutput=False)
    onescol = nc.declare_dram_parameter("onescol", [P, 1], f32r, isOutput=False)
    onespp = nc.declare_dram_parameter("onespp", [P, 1], f32, isOutput=False)
    y = nc.declare_dram_parameter("y", [S, D], f32, isOutput=True)

    def mm(out, lhsT, rhs, start, stop):
        nc.tensor.matmul(out, lhsT, rhs, start=start, stop=stop)

    def mm8(out, lhsT, rhs, start, stop):
        nc.tensor.matmul(out, lhsT, rhs, start=start, stop=stop, perf_mode=DR)

    with tile.TileContext(nc) as tc:
        es_top = ExitStack()

        consts = es_top.enter_context(tc.tile_pool(name="consts", bufs=1))
        mid = es_top.enter_context(tc.tile_pool(name="mid", bufs=1))
        rowp = es_top.enter_context(tc.tile_pool(name="rowp", bufs=1))
        rowbp = es_top.enter_context(tc.tile_pool(name="rowbp", bufs=2))
        scp = es_top.enter_context(tc.tile_pool(name="scp", bufs=2))
        es_qkv = ExitStack()
        xtp = es_qkv.enter_context(tc.tile_pool(name="xtp", bufs=1))
        qkvp = es_qkv.enter_context(tc.tile_pool(name="qkvp", bufs=1))
        es_x8 = ExitStack()
        x8p = es_x8.enter_context(tc.tile_pool(name="x8p", bufs=1))

        # ---- tiles (allocation only; DMA emission is scheduled below) --------
        on1b = consts.tile([1, P], bf16, tag="on1b")
        onc = consts.tile([P, 1], f32r, tag="onc")
        onpp = consts.tile([P, 1], f32, tag="onpp")
        eps_t = consts.tile([1, 1], f32, tag="eps")
        qb_sb = consts.tile([P, 8], f32, tag="qb")
        kb_sb = consts.tile([P, 8], f32, tag="kb")
        f1b_sb = consts.tile([P, 32], f32, tag="f1b")
        f2b_sb = consts.tile([P, 8], f32, tag="f2b")
        gb1 = consts.tile([P, 8], f32, tag="gb1")
        bb1 = consts.tile([P, 8], f32, tag="bb1")
        gb2 = consts.tile([P, 8], f32, tag="gb2")
        bb2 = consts.tile([P, 8], f32, tag="bb2")
        vb_row = consts.tile([1, D], bf16, tag="vbrow")
        pb_row = consts.tile([1, D], bf16, tag="pbrow")

        CT = mid.tile([P, 8, S], f32r, tag="ctff")     # ctx+resid, later FFT
        SQ = rowp.tile([P, 8, 512], bf16, tag="sq")    # squares (per c-half)
        O1T = mid.tile([P, 8, S], bf16, tag="o1t")
        XT = xtp.tile([P, 8, S], bf16, tag="xt")
        XT8 = x8p.tile([P, 8, S], f8, tag="xt8")
        QT = qkvp.tile([P, 8, S], bf16, tag="qt")
        KT = qkvp.tile([P, 8, S], bf16, tag="kt")
        Vp = qkvp.tile([P, 8, H * (DH + 1)], f8, tag="vp")
        Vp5 = Vp[:].rearrange("p i (hh e) -> p i hh e", e=DH + 1)

        # critical-path DMAs first: the first stationary weight tile, then XT
        xTr = xT[:].rearrange("(ko p) s -> p ko s", p=P)
        xT8r = xT8[:].rearrange("(ko p) s -> p ko s", p=P)

        es_ph1 = ExitStack()
        w1p = es_ph1.enter_context(tc.tile_pool(name="w1p", bufs=3))
        wj0q = w1p.tile([P, 8, P], f8, tag="wqk", name="wj0q")
        nc.sync.dma_start(wj0q[:], wq[0])
        for k in range(8):
            nc.sync.dma_start(XT8[:, k, :], xT8r[:, k, :])
        wj0k = w1p.tile([P, 8, P], f8, tag="wqk", name="wj0k")
        nc.sync.dma_start(wj0k[:], wk[0])
        wj1q = w1p.tile([P, 8, P], f8, tag="wqk", name="wj1q")
        nc.sync.dma_start(wj1q[:], wq[1])
        nc.sync.dma_start(qb_sb[:], qb[:].rearrange("(j p) -> p j", p=P))
        nc.sync.dma_start(kb_sb[:], kb[:].rearrange("(j p) -> p j", p=P))
        wvp = es_ph1.enter_context(tc.tile_pool(name="wvp", bufs=1))
        etp = es_ph1.enter_context(tc.tile_pool(name="etp", bufs=2))
        drp = es_ph1.enter_context(tc.tile_pool(name="drp", bufs=1))
        ps_sp = es_ph1.enter_context(
            tc.tile_pool(name="ps_sp", bufs=2, space="PSUM"))
        ps_cp = es_ph1.enter_context(
            tc.tile_pool(name="ps_cp", bufs=2, space="PSUM"))
        es_qkps = ExitStack()
        ps_qk = es_qkps.enter_context(
            tc.tile_pool(name="ps_qk", bufs=1, space="PSUM"))

        def emit_qk(j, pre=None):
            """Q and K projections for feature block j (fp8 DoubleRow over
            k-pairs).  psum pair tile: c0 -> bank 0, c1 -> bank 1, one fused
            relu evac writing 32*Q (resp 32*K) as fp8."""
            for wi, (wdram, bias_sb, out) in enumerate(
                    ((wq, qb_sb, QT), (wk, kb_sb, KT))):
                if pre is not None and pre[wi] is not None:
                    wj = pre[wi]
                else:
                    wj = w1p.tile([P, 8, P], f8, tag="wqk")
                    nc.sync.dma_start(wj[:], wdram[j])
                pt = ps_qk.tile([P, S], f32, tag="pqk")
                for c in range(2):
                    for t in range(4):
                        mm8(pt[:, c * 512:(c + 1) * 512],
                            wj[:, 2 * t:2 * t + 2, :],
                            XT8[:, 2 * t:2 * t + 2, c * 512:(c + 1) * 512],
                            start=(t == 0), stop=(t == 3))
                # relu(x+32b) on DVE: keeps phase-A ACT nearly exp-only
                nc.vector.tensor_scalar(out[:, j, :], pt[:],
                                        bias_sb[:, j:j + 1], 0.0,
                                        ALU.add, ALU.max)

        def emit_v(c):
            """V projection for dout half c (heads 8c..8c+7), fp8 DoubleRow."""
            wvc = wvp.tile([P, 8, 512], f8, tag="wvc")
            for k in range(8):
                nc.sync.dma_start(wvc[:, k, :], wv[:, k, c * 512:(c + 1) * 512])
            for i2 in range(4):
                pv = ps_qk.tile([P, S], f32, tag="pqk")
                for io in range(2):
                    i = 2 * i2 + io
                    hv = slice(io * 512, (io + 1) * 512)
                    for t in range(4):
                        mm8(pv[:, hv],
                            XT8[:, 2 * t:2 * t + 2, i * 128:(i + 1) * 128],
                            wvc[:, 2 * t:2 * t + 2, :],
                            start=(t == 0), stop=False)
                    mm(pv[:, hv], on1b[:], vb_row[:, c * 512:(c + 1) * 512],
                       start=False, stop=True)
                pv4 = pv[:].rearrange("p (io hh e) -> p io hh e", io=2, e=DH)
                nc.scalar.activation(
                    Vp5[:, 2 * i2:2 * i2 + 2, c * 8:(c + 1) * 8, 0:DH],
                    pv4[:], AF.Relu)

        def emit_attn(j, cset=(0, 1), cp_pool=None, filler=None):
            """Attention for head pair (2j, 2j+1).

            scores: bf16 row-tiled (u pairs at bases 0/64) into a 2-bank
            [P, 2, 512] psum per (t-pair, head); one exp per head reads
            [P, 2, 512] N=1024 and writes an fp8 et tile; ctx:
            [128,2,65]x[128,2,512] fp8 DoubleRow over t-pairs into a
            1-bank [65,512] psum.  `filler()` (if given) is called once
            per t-pair to splice independent PE work (FF1 blocks) into
            the queue so the exp stream never starves the PE."""
            cpp = cp_pool or ps_cp
            for c in cset:
                cs = slice(c * 512, (c + 1) * 512)
                cps = [cpp.tile([65, 512], f32, tag="cp",
                                name=f"cp_{j}_{c}_{u}") for u in range(2)]
                for tp in range(4):
                    # two 2-bank psums per t-pair (one per head) allocated
                    # up front, writes interleaved: the row-tiled score
                    # pair (bases 0/64) is adjacent in the PE queue so HW
                    # overlaps it, and the staggered exp completions match
                    # the staggered slot-reuse order of the next t-pair
                    sps = [ps_sp.tile([P, 2, 512], f32, tag="sp",
                                      name=f"sp{j}_{c}_{tp}_{u}")
                           for u in range(2)]
                    for i in range(2):
                        t = 2 * tp + i
                        for u in range(2):
                            r0 = 64 * u
                            mm(sps[u][:, i, :],
                               KT[r0:r0 + 64, j, t * 128:(t + 1) * 128],
                               QT[r0:r0 + 64, j, cs], start=True, stop=True)
                    for u in range(2):
                        et = etp.tile([P, 2, 512], f8, tag="et")
                        nc.scalar.activation(et[:], sps[u][:], AF.Exp,
                                             scale=EXPSC)
                        mm8(cps[u][:], Vp5[:, 2 * tp:2 * tp + 2, 2 * j + u, :],
                            et[:], start=(tp == 0), stop=(tp == 3))
                    if filler is not None:
                        filler()
                # normalize by denominator row + write CT (true scale: the
                # x32 of V cancels against the 32-valued ones column)
                for u in range(2):
                    dr = drp.tile([1, 512], f32, tag="dr")
                    nc.vector.reciprocal(dr[:], cps[u][64:65, :])
                    db = drp.tile([64, 512], f32, tag="db")
                    nc.gpsimd.partition_broadcast(db[:], dr[:], channels=64)
                    r0 = 64 * u
                    nc.vector.tensor_tensor(CT[r0:r0 + 64, j, cs],
                                            cps[u][0:64, :], db[:], ALU.mult)

        # ------- LayerNorm building blocks (transposed layout) ----------------
        def emit_resid(dst, other, j, cs, eng=None):
            (eng or nc.vector).tensor_tensor(dst[:, j, cs], dst[:, j, cs],
                                             other[:, j, cs], ALU.add)

        def emit_sq(c, src, js, eng):
            cs = slice(c * 512, (c + 1) * 512)
            for j in js:
                eng.tensor_tensor(SQ[:, j, :], src[:, j, cs], src[:, j, cs],
                                  ALU.mult)

        def emit_stats(ln_ps, c, src, nm):
            cs = slice(c * 512, (c + 1) * 512)
            psS = ln_ps.tile([1, 512], f32, tag="sums", name=f"psS_{nm}_{c}")
            psQ = ln_ps.tile([1, 512], f32, tag="sumq", name=f"psQ_{nm}_{c}")
            for j in range(8):
                mm(psS[:], onc[:], src[:, j, cs], start=(j == 0), stop=(j == 7))
                mm(psQ[:], onc[:], SQ[:, j, :], start=(j == 0), stop=(j == 7))
            return psS, psQ

        def emit_finalize(psS, psQ):
            """mean/var -> alpha (=1/std) and r2 (=mu/std), broadcast rows."""
            mu = rowp.tile([1, 512], f32, tag="mu")
            nc.scalar.activation(mu[:], psS[:], AF.Copy, scale=1.0 / D)
            ex2 = rowp.tile([1, 512], f32, tag="ex2")
            nc.scalar.activation(ex2[:], psQ[:], AF.Copy, scale=1.0 / D)
            var = rowp.tile([1, 512], f32, tag="var")
            nc.vector.tensor_tensor(var[:], mu[:], mu[:], ALU.mult)
            nc.vector.tensor_tensor(var[:], ex2[:], var[:], ALU.subtract)
            al = rowp.tile([1, 512], f32, tag="al")
            nc.scalar.activation(al[:], var[:], AF.Sqrt, bias=eps_t[:])
            nc.vector.reciprocal(al[:], al[:])
            r2 = rowp.tile([1, 512], f32, tag="r2")
            nc.vector.tensor_tensor(r2[:], mu[:], al[:], ALU.mult)
            ab = rowbp.tile([P, 512], f32, tag="ab")
            nc.gpsimd.partition_broadcast(ab[:], al[:], channels=P)
            rb = rowbp.tile([P, 512], f32, tag="rb")
            nc.gpsimd.partition_broadcast(rb[:], r2[:], channels=P)
            return ab, rb

        def emit_apply(c, src, gcol, bcol, out, ab, rb, dve_js, js=tuple(range(8))):
            """out = (src*alpha - r2)*g + b; split across DVE and Pool."""
            cs = slice(c * 512, (c + 1) * 512)
            for j in js:
                if j in dve_js:
                    sc = scp.tile([P, 512], f32, tag="scv")
                    nc.vector.tensor_tensor(sc[:], src[:, j, cs], ab[:],
                                            ALU.mult)
                    nc.vector.tensor_tensor(sc[:], sc[:], rb[:], ALU.subtract)
                    nc.vector.tensor_scalar(out[:, j, cs], sc[:],
                                            gcol[:, j:j + 1], bcol[:, j:j + 1],
                                            ALU.mult, ALU.add)
                else:
                    sc = scp.tile([P, 512], f32, tag="scp")
                    nc.gpsimd.tensor_tensor(sc[:], src[:, j, cs], ab[:],
                                            ALU.mult)
                    nc.gpsimd.tensor_tensor(sc[:], sc[:], rb[:], ALU.subtract)
                    nc.gpsimd.tensor_scalar(out[:, j, cs], sc[:],
                                            gcol[:, j:j + 1], bcol[:, j:j + 1],
                                            ALU.mult, ALU.add)

        DVE_JS = (0, 1, 2)   # Pool is faster per op; give it the bigger share

        # ---- phase A: QKV + attention, interleaved ---------------------------
        emit_qk(0, pre=(wj0q, wj0k))
        # small consts stream in behind the first weight loads
        nc.sync.dma_start(onpp[:], onespp[:])
        nc.sync.dma_start(on1b[:], ones1b[:])
        nc.sync.dma_start(vb_row[:], vb[None, :])
        # softmax-denominator ones column, value 32 (cancels V's x32)
        vp_col = Vp[:].rearrange("p i (hh e) -> p (i hh) e", e=DH + 1)[:, :, DH]
        nc.scalar.activation(vp_col, onpp[:].to_broadcast((P, 8 * H)), AF.Copy,
                             scale=WSC)
        emit_qk(1, pre=(wj1q, None))
        nc.sync.dma_start(onc[:], onescol[:])
        nc.vector.memset(eps_t[:], EPS)
        nc.sync.dma_start(gb1[:], lng[:].rearrange("(j p) -> p j", p=P))
        nc.sync.dma_start(bb1[:], lnb[:].rearrange("(j p) -> p j", p=P))
        nc.sync.dma_start(gb2[:], fflng[:].rearrange("(j p) -> p j", p=P))
        nc.sync.dma_start(bb2[:], fflnb[:].rearrange("(j p) -> p j", p=P))
        nc.sync.dma_start(f1b_sb[:], f1b[:].rearrange("(j p) -> p j", p=P))
        nc.sync.dma_start(f2b_sb[:], f2b[:].rearrange("(j p) -> p j", p=P))
        nc.sync.dma_start(pb_row[:], pb[None, :])
        # bf16 XT (residual path only) streams behind the fp8 critical path
        for k in range(8):
            nc.sync.dma_start(XT[:, k, :], xTr[:, k, :])
        emit_v(0)
        emit_qk(2)
        # ---- c0 sweep: attention on queries 0..511 for all pairs, QKV
        # projections for later blocks interleaved behind the exp stream.
        emit_attn(0, (0,))
        emit_qk(3)
        emit_resid(CT, XT, 0, slice(0, 512))
        emit_sq(0, CT, (0,), nc.gpsimd)
        emit_v(1)
        emit_attn(1, (0,))
        emit_qk(4)
        emit_resid(CT, XT, 1, slice(0, 512))
        emit_sq(0, CT, (1,), nc.gpsimd)
        emit_attn(2, (0,))
        emit_qk(5)
        emit_resid(CT, XT, 2, slice(0, 512))
        emit_sq(0, CT, (2,), nc.gpsimd)
        emit_attn(3, (0,))
        emit_qk(6)
        emit_resid(CT, XT, 3, slice(0, 512))
        emit_sq(0, CT, (3,), nc.gpsimd)
        emit_attn(4, (0,))
        emit_qk(7)
        emit_resid(CT, XT, 4, slice(0, 512))
        emit_sq(0, CT, (4,), nc.gpsimd)
        emit_attn(5, (0,))
        emit_resid(CT, XT, 5, slice(0, 512))
        emit_sq(0, CT, (5,), nc.gpsimd)
        emit_attn(6, (0,))
        emit_resid(CT, XT, 6, slice(0, 512))
        emit_sq(0, CT, (6,), nc.gpsimd)
        emit_attn(7, (0,))
        emit_resid(CT, XT, 7, slice(0, 512))
        emit_sq(0, CT, (7,), nc.gpsimd)
        # QK/V psum no longer needed; swap those banks to the LN1-c0 stats
        es_qkps.close()
        es_lnA = ExitStack()
        ln_psA = es_lnA.enter_context(
            tc.tile_pool(name="ln_psA", bufs=1, space="PSUM"))
        psS0, psQ0 = emit_stats(ln_psA, 0, CT, "ln1")
        ab0, rb0 = emit_finalize(psS0, psQ0)
        es_lnA.close()
        # apply-c0 split DVE/Pool: the first FF1 filler block gates on it
        emit_apply(0, CT, gb1, bb1, O1T, ab0, rb0, (0, 1, 2))

        # ---- overlap window: attention c1 (exp-bound on ACT) carries the
        # FF1-c0 matmuls as PE filler, one m-block per t-pair.
        es_x8.close()   # free XT8 (projections done)
        es_ffa = ExitStack()
        ffap = es_ffa.enter_context(tc.tile_pool(name="ffap", bufs=1))
        wf1p = es_ffa.enter_context(tc.tile_pool(name="wf1p", bufs=3))
        ff_ps = es_ffa.enter_context(
            tc.tile_pool(name="ff_ps", bufs=2, space="PSUM"))
        H1 = ffap.tile([P, 32, 512], bf16, tag="h1")
        FFT = mid.tile([P, 8, S], f32r, tag="ctff")  # reuse CT buffer

        def emit_ff1_block(m, c, relu_on_act):
            cs = slice(c * 512, (c + 1) * 512)
            wm = wf1p.tile([P, 8, P], bf16, tag="wf1")
            nc.sync.dma_start(wm[:], wf1[m])
            pt = ff_ps.tile([P, 512], f32, tag="ff")
            for k in range(8):
                mm(pt[:], wm[:, k, :], O1T[:, k, cs],
                   start=(k == 0), stop=(k == 7))
            if relu_on_act:
                nc.scalar.activation(H1[:, m, :], pt[:], AF.Relu,
                                     bias=f1b_sb[:, m:m + 1])
            else:
                # DVE relu evac: keeps the overlap window's ACT exp-only
                nc.vector.tensor_scalar(H1[:, m, :], pt[:],
                                        f1b_sb[:, m:m + 1], 0.0,
                                        ALU.add, ALU.max)

        ff1_m = iter(range(32))

        def ff1_filler():
            m = next(ff1_m, None)
            if m is not None:
                emit_ff1_block(m, 0, relu_on_act=False)

        for j in range(8):
            emit_attn(j, (1,), filler=ff1_filler)
            emit_resid(CT, XT, j, slice(512, 1024))
            emit_sq(1, CT, (j,), nc.gpsimd)
        for m in ff1_m:  # any filler slots the attention loop didn't consume
            emit_ff1_block(m, 0, relu_on_act=False)

        es_ph1.close()
        es_qkv.close()   # free XT / XT8 / QT / KT / Vp

        es_ph2 = ExitStack()
        ln_ps1 = es_ph2.enter_context(
            tc.tile_pool(name="ln_ps1", bufs=1, space="PSUM"))

        # LN1-c1 chain; the FF2-c0 matmuls right after keep the PE busy
        # while finalize/apply run on ACT/DVE/Pool.
        psS1, psQ1 = emit_stats(ln_ps1, 1, CT, "ln1")
        ab1, rb1 = emit_finalize(psS1, psQ1)
        emit_apply(1, CT, gb1, bb1, O1T, ab1, rb1, (0, 1, 2))

        # ---- phase C pools (FF + LN2 + proj) ---------------------------------
        es_ph3 = ExitStack()
        ffp = es_ph3.enter_context(tc.tile_pool(name="ffp", bufs=1))
        wf2p = es_ph3.enter_context(tc.tile_pool(name="wf2p", bufs=2))
        ytp = es_ph3.enter_context(tc.tile_pool(name="ytp", bufs=3))
        pj_ps = es_ph3.enter_context(
            tc.tile_pool(name="pj_ps", bufs=2, space="PSUM"))

        O2T = ffp.tile([P, 8, S], bf16, tag="o2t")
        WP = ffp.tile([P, 8, D], bf16, tag="wp")

        def emit_ff1(c):
            for m in range(32):
                emit_ff1_block(m, c, relu_on_act=True)

        def emit_ff2(c, pre=()):
            cs = slice(c * 512, (c + 1) * 512)
            for j in range(8):
                if j < len(pre):
                    w2j = pre[j]
                else:
                    w2j = wf2p.tile([P, 32, P], bf16, tag="w2j")
                    nc.sync.dma_start(w2j[:], wf2[j])
                pt = ff_ps.tile([P, 512], f32, tag="ff")
                for m in range(32):
                    mm(pt[:], w2j[:, m, :], H1[:, m, :],
                       start=(m == 0), stop=(m == 31))
                # fused evac: FFT = (psum + f2b) + O1T  (bias + residual)
                nc.vector.scalar_tensor_tensor(
                    FFT[:, j, cs], pt[:], f2b_sb[:, j:j + 1],
                    O1T[:, j, cs], ALU.add, ALU.add)

        def emit_proj(iset, split_last=False):
            for i in iset:
                yt = ytp.tile([P, D], f32, tag="yt")
                pp = pj_ps.tile([P, D], f32, tag="pj")
                split = split_last and i == iset[-1]
                for dh in range(2):
                    ds_ = slice(dh * 512, (dh + 1) * 512)
                    for k in range(8):
                        mm(pp[:, ds_], O2T[:, k, i * 128:(i + 1) * 128],
                           WP[:, k, ds_], start=(k == 0), stop=False)
                    mm(pp[:, ds_], on1b[:], pb_row[:, ds_],
                       start=False, stop=True)
                    if split:
                        nc.scalar.activation(yt[:, ds_], pp[:, ds_], AF.Copy)
                        nc.sync.dma_start(y[i * 128:(i + 1) * 128, ds_],
                                          yt[:, ds_])
                if not split:
                    nc.scalar.activation(yt[:], pp[:], AF.Copy)
                    nc.sync.dma_start(y[i * 128:(i + 1) * 128, :], yt[:])

        nc.sync.dma_start(WP[:], wp[:])
        emit_ff2(0)
        emit_ff1(1)
        # LN2 c0: chain overlaps FF1 c1 matmuls (residual fused into FF2 evac)
        emit_sq(0, FFT, tuple(range(8)), nc.gpsimd)
        psS2, psQ2 = emit_stats(ln_ps1, 0, FFT, "ln2")
        ab2, rb2 = emit_finalize(psS2, psQ2)
        emit_apply(0, FFT, gb2, bb2, O2T, ab2, rb2, (0, 1))
        emit_ff2(1)
        # LN2 c1 chain overlaps proj i0-i1 (residual fused into FF2 evac)
        emit_sq(1, FFT, tuple(range(8)), nc.gpsimd)
        emit_proj((0, 1))
        psS3, psQ3 = emit_stats(ln_ps1, 1, FFT, "ln2")
        ab3, rb3 = emit_finalize(psS3, psQ3)
        emit_proj((2, 3))
        emit_apply(1, FFT, gb2, bb2, O2T, ab3, rb3, DVE_JS)
        emit_proj((4, 5))
        emit_proj((6,), split_last=True)
        emit_proj((7,), split_last=True)

        es_ph3.close()
        es_ffa.close()
        es_ph2.close()
        es_top.close()

    nc.compile()
    return nc


# ---------------------------------------------------------------- host wrapper
class _SpmdRunner:
    """Compile once, run repeatedly (mirrors bass2jax.run_bass_via_pjrt)."""

    def __init__(self, nc, n_cores):
        import jax
        from jax.sharding import Mesh, PartitionSpec
        from jax.experimental.shard_map import shard_map
        import concourse.mybir as mybir
        from concourse import bass2jax
        from concourse.bass2jax import _bass_exec_p, install_neuronx_cc_hook

        install_neuronx_cc_hook()
        self.n_cores = n_cores
        partition_name = (
            nc.partition_id_tensor.name if nc.partition_id_tensor else None
        )
        in_names, out_names, out_avals, zero_outs = [], [], [], []
        for alloc in nc.m.functions[0].allocations:
            if not isinstance(alloc, mybir.MemoryLocationSet):
                continue
            name = alloc.memorylocations[0].name
            if alloc.kind == "ExternalInput":
                if name != partition_name:
                    in_names.append(name)
            elif alloc.kind == "ExternalOutput":
                shape = tuple(alloc.tensor_shape)
                dtype = mybir.dt.np(alloc.dtype)
                out_names.append(name)
                out_avals.append(jax.core.ShapedArray(shape, dtype))
                zero_outs.append(np.zeros(shape, dtype))
        self.in_names = in_names
        self.out_names = out_names
        self.out_avals = out_avals
        self.zero_outs = zero_outs
        n_params = len(in_names)
        n_outs = len(out_avals)
        all_in_names = in_names + out_names
        if partition_name is not None:
            all_in_names.append(partition_name)
        donate = tuple(range(n_params, n_params + n_outs))

        def _body(*args):
            operands = list(args)
            if partition_name is not None:
                operands.append(bass2jax.partition_id_tensor())
            outs = _bass_exec_p.bind(
                *operands,
                out_avals=tuple(out_avals),
                in_names=tuple(all_in_names),
                out_names=tuple(out_names),
                lowering_input_output_aliases=(),
                sim_require_finite=True,
                sim_require_nnan=True,
                nc=nc,
            )
            return tuple(outs)

        import jax as _jax
        devices = _jax.devices()[:n_cores]
        assert len(devices) == n_cores
        mesh = Mesh(np.asarray(devices), ("core",))
        in_specs = (PartitionSpec("core"),) * (n_params + n_outs)
        out_specs = (PartitionSpec("core"),) * n_outs
        self.fn = _jax.jit(
            shard_map(_body, mesh=mesh, in_specs=in_specs,
                      out_specs=out_specs, check_rep=False),
            donate_argnums=donate,
            keep_unused=True,
        )

    def prep_inputs(self, in_maps):
        per_core = [[np.asarray(m[n]) for n in self.in_names] for m in in_maps]
        return [
            np.concatenate([per_core[c][i] for c in range(self.n_cores)], axis=0)
            for i in range(len(self.in_names))
        ]

    def zeros(self):
        return [
            np.zeros((self.n_cores * z.shape[0], *z.shape[1:]), z.dtype)
            for z in self.zero_outs
        ]

    def run_device(self, concat_in):
        return self.fn(*concat_in, *self.zeros())

    def split(self, out_arrs):
        return [
            {
                name: np.asarray(out_arrs[i]).reshape(
                    self.n_cores, *self.out_avals[i].shape)[c]
                for i, name in enumerate(self.out_names)
            }
            for c in range(self.n_cores)
        ]


def make_in_maps(**inputs):
    import ml_dtypes
    BF16 = np.dtype(ml_dtypes.bfloat16)
    F8 = np.dtype(ml_dtypes.float8_e4m3)
    f32 = np.float32
    q = np.ascontiguousarray(np.asarray(inputs["queries"], dtype=f32))

    def arr(name):
        return np.ascontiguousarray(np.asarray(inputs[name], dtype=f32))

    Qw, Kw, Vw = arr("Qw"), arr("Kw"), arr("Vw")
    proj_w, ff1_w, ff2_w = arr("proj_w"), arr("ff1_w"), arr("ff2_w")

    # packed weight layouts (all-contiguous device DMAs)
    def pack_lhsT(w, nj):  # [dout, din] -> [j, p(k), ko, mc]
        return np.ascontiguousarray(
            w.reshape(nj, P, 8, P).transpose(0, 3, 2, 1))

    def pack_rhs(w):  # [dout, din] -> W^T as [p(k), ko, dout]
        return np.ascontiguousarray(
            w.T.reshape(8, P, w.shape[0]).transpose(1, 0, 2))

    shared = {
        "wq": pack_lhsT(Qw * WSC, 8).astype(F8),
        "wk": pack_lhsT(Kw * WSC, 8).astype(F8),
        "wv": np.ascontiguousarray(
            pack_rhs(Vw * WSC).reshape(P, 8, 2, 512).transpose(2, 0, 1, 3)
        ).astype(F8),
        "wp": pack_rhs(proj_w).astype(BF16),
        "wf1": pack_lhsT(ff1_w, 32).astype(BF16),
        "wf2": np.ascontiguousarray(
            ff2_w.T.reshape(32, P, 8, P).transpose(2, 1, 0, 3)).astype(BF16),
        "qb": arr("Qb") * WSC, "kb": arr("Kb") * WSC,
        "vb": (arr("Vb") * WSC).astype(BF16),
        "f1b": arr("ff1_b"), "f2b": arr("ff2_b"),
        "pb": arr("proj_b").astype(BF16),
        "lng": arr("ln_g"), "lnb": arr("ln_b"),
        "fflng": arr("ffln_g"), "fflnb": arr("ffln_b"),
        "ones1b": np.ones((1, P), dtype=f32).astype(BF16),
        "onescol": np.ones((P, 1), dtype=f32),
        "onespp": np.ones((P, 1), dtype=f32),
    }
    in_maps = []
    for b in range(B):
        m = dict(shared)
        xb = np.ascontiguousarray(
            q[b].T.reshape(8, P, S).transpose(1, 0, 2))
        m["xT"] = xb.astype(BF16)
        m["xT8"] = xb.astype(F8)
        in_maps.append(m)
    return in_maps


def get_runner():
    global _RUNNER
    if _RUNNER is None:
        nc = build_nc()
        _RUNNER = _SpmdRunner(nc, NCORES)
    return _RUNNER


def kernel(**inputs):
    runner = get_runner()
    in_maps = make_in_maps(**inputs)
    res = runner.split(runner.run_device(runner.prep_inputs(in_maps)))
    out = np.stack([res[c]["y"] for c in range(NCORES)], axis=0)
    return out.astype(np.float32)


# revision 46
# speedup vs baseline: 1.5348x; 1.1982x over previous
"""Trainium2 Bass kernel for nn_MultiHead (dense transformer layer).

Strategy: pure data-parallel over batch (B=8 -> 8 NeuronCores, no collectives).
Per core: full transformer layer on one [S=1024, D=1024] batch element.

v6 design (on top of the v3 fully-transposed layout):
  - both attention sweeps run behind per-t-pair PE fillers: the c0 sweep
    consumes the remaining QKV projections as fine-grained units (one
    psum group each, just-in-time before the chunk that needs them), and
    the c1 sweep consumes all 32 FF1-c0 m-blocks; the in-order PE queue
    therefore always has ready work at the exp-pipeline stall points.
    The two row-tiled score matmuls of each t-block (PE row groups 0/64)
    are emitted back-to-back into per-head psums so the hardware runs
    them concurrently (the cost model serializes them; hardware does
    not -- worth ~27us there).
  - c-split software pipeline: attention runs queries 0-511 for all 16
    heads first (ACT exp-bound), then LN1-c0, then the c1 attention
    sweep carries all 32 FF1-c0 m-blocks as per-t-pair PE filler inside
    emit_attn -- the exp stream and the FF matmuls share the window, so
    the PE stays ~100% busy from LN1-c0 to the end of the kernel.
  - fp8e4 (e4m3) + DoubleRow perf mode for the Q/K/V projections and the
    ctx accumulation (2x PE rate).  Scores stay bf16 (DoubleRow there
    would need 32-row tiles at base partition 96, which the HW forbids);
    FF1/FF2/proj stay bf16 (fp8 there breaks the 2e-2 gate: measured
    relmax ~1.9e-2 per site in an offline quantization study).
  - fp8 range handling: weights are pre-scaled x32 host-side so w~0.02
    values sit in e4m3's normal range; activations (Q/K/V x32, x true
    scale) stay well under the 240 saturation limit.  The x32 scales
    cancel: QK evac adds 32*bias (Q,K stored as 32*Q, bf16), the exp
    scale absorbs 1/1024, and the softmax denominator ones-column is 32
    so the normalize restores true ctx.
  - ctx DoubleRow pairs t-blocks: the score psum is a 4-bank [P, 2, S]
    tile per t-pair (4 bf16 row-tiled matmuls), one exp per head reads
    [P, 2, 512] N=1024 and writes a [P, 2, 512] fp8 et tile, and the
    ctx DR matmul contracts both t-blocks against the [128, 2, 65]
    V (+32*ones col) stationary into a 1-bank [65, 512] psum.
  - LayerNorm stats / softmax denominators / residuals / FF unchanged
    from v3 (ones-column matmuls, f32r LN inputs, bf16 FF).
"""
from contextlib import ExitStack

import numpy as np

S = 1024
D = 1024
H = 16
DH = 64
DFF = 4096
P = 128
B = 8
NCORES = 8
EPS = 1e-8
WSC = 32.0           # fp8 weight pre-scale
EXPSC = 0.125 / (WSC * WSC)  # exp scale: 1/sqrt(DH) / (32*32)

_RUNNER = None


# ---------------------------------------------------------------- device kernel
def build_nc():
    import concourse.mybir as mybir
    import concourse.tile as tile
    from concourse import bacc

    f32 = mybir.dt.float32
    f32r = mybir.dt.float32r
    bf16 = mybir.dt.bfloat16
    f8 = mybir.dt.float8e4
    AF = mybir.ActivationFunctionType
    ALU = mybir.AluOpType
    DR = mybir.MatmulPerfMode.DoubleRow

    nc = bacc.Bacc("TRN2", target_bir_lowering=False, debug=False)

    # ---- I/O -----------------------------------------------------------------
    xT = nc.declare_dram_parameter("xT", [P, 8, S], bf16, isOutput=False)
    xT8 = nc.declare_dram_parameter("xT8", [P, 8, S], f8, isOutput=False)
    wq = nc.declare_dram_parameter("wq", [8, P, 8, P], f8, isOutput=False)
    wk = nc.declare_dram_parameter("wk", [8, P, 8, P], f8, isOutput=False)
    wv = nc.declare_dram_parameter("wv", [2, P, 8, 512], f8, isOutput=False)
    wp = nc.declare_dram_parameter("wp", [P, 8, D], bf16, isOutput=False)
    wf1 = nc.declare_dram_parameter("wf1", [32, P, 8, P], bf16, isOutput=False)
    wf2 = nc.declare_dram_parameter("wf2", [8, P, 32, P], bf16, isOutput=False)
    qb = nc.declare_dram_parameter("qb", [D], f32, isOutput=False)   # 32x, perm
    kb = nc.declare_dram_parameter("kb", [D], f32, isOutput=False)   # 32x, perm
    vb = nc.declare_dram_parameter("vb", [D], bf16, isOutput=False)  # 32x
    f1b = nc.declare_dram_parameter("f1b", [DFF], f32, isOutput=False)
    f2b = nc.declare_dram_parameter("f2b", [D], f32, isOutput=False)
    pb = nc.declare_dram_parameter("pb", [D], bf16, isOutput=False)
    lng = nc.declare_dram_parameter("lng", [D], f32, isOutput=False)
    lnb = nc.declare_dram_parameter("lnb", [D], f32, isOutput=False)
    fflng = nc.declare_dram_parameter("fflng", [D], f32, isOutput=False)
    fflnb = nc.declare_dram_parameter("fflnb", [D], f32, isOutput=False)
    ones1b = nc.declare_dram_parameter("ones1b", [1, P], bf16, isOutput=False)
    onescol = nc.declare_dram_parameter("onescol", [P, 1], f32r, isOutput=False)
    onespp = nc.declare_dram_parameter("onespp", [P, 1], f32, isOutput=False)
    y = nc.declare_dram_parameter("y", [S, D], f32, isOutput=True)

    def mm(out, lhsT, rhs, start, stop):
        nc.tensor.matmul(out, lhsT, rhs, start=start, stop=stop)

    def mm8(out, lhsT, rhs, start, stop):
        nc.tensor.matmul(out, lhsT, rhs, start=start, stop=stop, perf_mode=DR)

    with tile.TileContext(nc) as tc:
        es_top = ExitStack()

        consts = es_top.enter_context(tc.tile_pool(name="consts", bufs=1))
        mid = es_top.enter_context(tc.tile_pool(name="mid", bufs=1))
        rowp = es_top.enter_context(tc.tile_pool(name="rowp", bufs=1))
        rowbp = es_top.enter_context(tc.tile_pool(name="rowbp", bufs=2))
        scp = es_top.enter_context(tc.tile_pool(name="scp", bufs=2))
        es_qkv = ExitStack()
        xtp = es_qkv.enter_context(tc.tile_pool(name="xtp", bufs=1))
        qkvp = es_qkv.enter_context(tc.tile_pool(name="qkvp", bufs=1))
        es_x8 = ExitStack()
        x8p = es_x8.enter_context(tc.tile_pool(name="x8p", bufs=1))

        # ---- tiles (allocation only; DMA emission is scheduled below) --------
        on1b = consts.tile([1, P], bf16, tag="on1b")
        onc = consts.tile([P, 1], f32r, tag="onc")
        onpp = consts.tile([P, 1], f32, tag="onpp")
        eps_t = consts.tile([1, 1], f32, tag="eps")
        qb_sb = consts.tile([P, 8], f32, tag="qb")
        kb_sb = consts.tile([P, 8], f32, tag="kb")
        f1b_sb = consts.tile([P, 32], f32, tag="f1b")
        f2b_sb = consts.tile([P, 8], f32, tag="f2b")
        gb1 = consts.tile([P, 8], f32, tag="gb1")
        bb1 = consts.tile([P, 8], f32, tag="bb1")
        gb2 = consts.tile([P, 8], f32, tag="gb2")
        bb2 = consts.tile([P, 8], f32, tag="bb2")
        vb_row = consts.tile([1, D], bf16, tag="vbrow")
        pb_row = consts.tile([1, D], bf16, tag="pbrow")

        CT = mid.tile([P, 8, S], f32r, tag="ctff")     # ctx+resid, later FFT
        SQ = rowp.tile([P, 8, 512], bf16, tag="sq")    # squares (per c-half)
        O1T = mid.tile([P, 8, S], bf16, tag="o1t")
        XT = xtp.tile([P, 8, S], bf16, tag="xt")
        XT8 = x8p.tile([P, 8, S], f8, tag="xt8")
        QT = qkvp.tile([P, 8, S], bf16, tag="qt")
        KT = qkvp.tile([P, 8, S], bf16, tag="kt")
        Vp = qkvp.tile([P, 8, H * (DH + 1)], f8, tag="vp")
        Vp5 = Vp[:].rearrange("p i (hh e) -> p i hh e", e=DH + 1)

        # critical-path DMAs first: the first stationary weight tile, then XT
        xTr = xT[:].rearrange("(ko p) s -> p ko s", p=P)
        xT8r = xT8[:].rearrange("(ko p) s -> p ko s", p=P)

        es_ph1 = ExitStack()
        w1p = es_ph1.enter_context(tc.tile_pool(name="w1p", bufs=3))
        wj0q = w1p.tile([P, 8, P], f8, tag="wqk", name="wj0q")
        nc.sync.dma_start(wj0q[:], wq[0])
        for k in range(8):
            nc.sync.dma_start(XT8[:, k, :], xT8r[:, k, :])
        wj0k = w1p.tile([P, 8, P], f8, tag="wqk", name="wj0k")
        nc.sync.dma_start(wj0k[:], wk[0])
        wj1q = w1p.tile([P, 8, P], f8, tag="wqk", name="wj1q")
        nc.sync.dma_start(wj1q[:], wq[1])
        nc.sync.dma_start(qb_sb[:], qb[:].rearrange("(j p) -> p j", p=P))
        nc.sync.dma_start(kb_sb[:], kb[:].rearrange("(j p) -> p j", p=P))
        wvp = es_ph1.enter_context(tc.tile_pool(name="wvp", bufs=1))
        etp = es_ph1.enter_context(tc.tile_pool(name="etp", bufs=2))
        drp = es_ph1.enter_context(tc.tile_pool(name="drp", bufs=1))
        ps_sp = es_ph1.enter_context(
            tc.tile_pool(name="ps_sp", bufs=2, space="PSUM"))
        ps_cp = es_ph1.enter_context(
            tc.tile_pool(name="ps_cp", bufs=2, space="PSUM"))
        es_qkps = ExitStack()
        ps_qk = es_qkps.enter_context(
            tc.tile_pool(name="ps_qk", bufs=1, space="PSUM"))

        def emit_qk(j, pre=None):
            """Q and K projections for feature block j (fp8 DoubleRow over
            k-pairs).  psum pair tile: c0 -> bank 0, c1 -> bank 1, one fused
            relu evac writing 32*Q (resp 32*K) as fp8."""
            for wi, (wdram, bias_sb, out) in enumerate(
                    ((wq, qb_sb, QT), (wk, kb_sb, KT))):
                if pre is not None and pre[wi] is not None:
                    wj = pre[wi]
                else:
                    wj = w1p.tile([P, 8, P], f8, tag="wqk")
                    nc.sync.dma_start(wj[:], wdram[j])
                pt = ps_qk.tile([P, S], f32, tag="pqk")
                for c in range(2):
                    for t in range(4):
                        mm8(pt[:, c * 512:(c + 1) * 512],
                            wj[:, 2 * t:2 * t + 2, :],
                            XT8[:, 2 * t:2 * t + 2, c * 512:(c + 1) * 512],
                            start=(t == 0), stop=(t == 3))
                # relu(x+32b) on DVE: keeps phase-A ACT nearly exp-only
                nc.vector.tensor_scalar(out[:, j, :], pt[:],
                                        bias_sb[:, j:j + 1], 0.0,
                                        ALU.add, ALU.max)

        def emit_v(c):
            """V projection for dout half c (heads 8c..8c+7), fp8 DoubleRow."""
            wvc = wvp.tile([P, 8, 512], f8, tag="wvc")
            for k in range(8):
                nc.sync.dma_start(wvc[:, k, :], wv[:, k, c * 512:(c + 1) * 512])
            for i2 in range(4):
                pv = ps_qk.tile([P, S], f32, tag="pqk")
                for io in range(2):
                    i = 2 * i2 + io
                    hv = slice(io * 512, (io + 1) * 512)
                    for t in range(4):
                        mm8(pv[:, hv],
                            XT8[:, 2 * t:2 * t + 2, i * 128:(i + 1) * 128],
                            wvc[:, 2 * t:2 * t + 2, :],
                            start=(t == 0), stop=False)
                    mm(pv[:, hv], on1b[:], vb_row[:, c * 512:(c + 1) * 512],
                       start=False, stop=True)
                pv4 = pv[:].rearrange("p (io hh e) -> p io hh e", io=2, e=DH)
                nc.scalar.activation(
                    Vp5[:, 2 * i2:2 * i2 + 2, c * 8:(c + 1) * 8, 0:DH],
                    pv4[:], AF.Relu)

        def emit_attn(j, cset=(0, 1), cp_pool=None, filler=None):
            """Attention for head pair (2j, 2j+1).

            scores: bf16 row-tiled (u pairs at bases 0/64) into a 2-bank
            [P, 2, 512] psum per (t-pair, head); one exp per head reads
            [P, 2, 512] N=1024 and writes an fp8 et tile; ctx:
            [128,2,65]x[128,2,512] fp8 DoubleRow over t-pairs into a
            1-bank [65,512] psum.  `filler()` (if given) is called once
            per t-pair to splice independent PE work (FF1 blocks) into
            the queue so the exp stream never starves the PE."""
            cpp = cp_pool or ps_cp
            for c in cset:
                cs = slice(c * 512, (c + 1) * 512)
                cps = [cpp.tile([65, 512], f32, tag="cp",
                                name=f"cp_{j}_{c}_{u}") for u in range(2)]
                for tp in range(4):
                    # two 2-bank psums per t-pair (one per head) allocated
                    # up front, writes interleaved: the row-tiled score
                    # pair (bases 0/64) is adjacent in the PE queue so HW
                    # overlaps it, and the staggered exp completions match
                    # the staggered slot-reuse order of the next t-pair
                    sps = [ps_sp.tile([P, 2, 512], f32, tag="sp",
                                      name=f"sp{j}_{c}_{tp}_{u}")
                           for u in range(2)]
                    for i in range(2):
                        t = 2 * tp + i
                        for u in range(2):
                            r0 = 64 * u
                            mm(sps[u][:, i, :],
                               KT[r0:r0 + 64, j, t * 128:(t + 1) * 128],
                               QT[r0:r0 + 64, j, cs], start=True, stop=True)
                    for u in range(2):
                        et = etp.tile([P, 2, 512], f8, tag="et")
                        nc.scalar.activation(et[:], sps[u][:], AF.Exp,
                                             scale=EXPSC)
                        mm8(cps[u][:], Vp5[:, 2 * tp:2 * tp + 2, 2 * j + u, :],
                            et[:], start=(tp == 0), stop=(tp == 3))
                    if filler is not None:
                        filler()
                # normalize by denominator row + write CT (true scale: the
                # x32 of V cancels against the 32-valued ones column)
                for u in range(2):
                    dr = drp.tile([1, 512], f32, tag="dr")
                    nc.vector.reciprocal(dr[:], cps[u][64:65, :])
                    db = drp.tile([64, 512], f32, tag="db")
                    nc.gpsimd.partition_broadcast(db[:], dr[:], channels=64)
                    r0 = 64 * u
                    nc.vector.tensor_tensor(CT[r0:r0 + 64, j, cs],
                                            cps[u][0:64, :], db[:], ALU.mult)

        # ------- LayerNorm building blocks (transposed layout) ----------------
        def emit_resid(dst, other, j, cs, eng=None):
            (eng or nc.vector).tensor_tensor(dst[:, j, cs], dst[:, j, cs],
                                             other[:, j, cs], ALU.add)

        def emit_sq(c, src, js, eng):
            cs = slice(c * 512, (c + 1) * 512)
            for j in js:
                eng.tensor_tensor(SQ[:, j, :], src[:, j, cs], src[:, j, cs],
                                  ALU.mult)

        def emit_stats(ln_ps, c, src, nm):
            cs = slice(c * 512, (c + 1) * 512)
            psS = ln_ps.tile([1, 512], f32, tag="sums", name=f"psS_{nm}_{c}")
            psQ = ln_ps.tile([1, 512], f32, tag="sumq", name=f"psQ_{nm}_{c}")
            for j in range(8):
                mm(psS[:], onc[:], src[:, j, cs], start=(j == 0), stop=(j == 7))
                mm(psQ[:], onc[:], SQ[:, j, :], start=(j == 0), stop=(j == 7))
            return psS, psQ

        def emit_finalize(psS, psQ):
            """mean/var -> alpha (=1/std) and r2 (=mu/std), broadcast rows."""
            mu = rowp.tile([1, 512], f32, tag="mu")
            nc.scalar.activation(mu[:], psS[:], AF.Copy, scale=1.0 / D)
            ex2 = rowp.tile([1, 512], f32, tag="ex2")
            nc.scalar.activation(ex2[:], psQ[:], AF.Copy, scale=1.0 / D)
            var = rowp.tile([1, 512], f32, tag="var")
            nc.vector.tensor_tensor(var[:], mu[:], mu[:], ALU.mult)
            nc.vector.tensor_tensor(var[:], ex2[:], var[:], ALU.subtract)
            al = rowp.tile([1, 512], f32, tag="al")
            nc.scalar.activation(al[:], var[:], AF.Sqrt, bias=eps_t[:])
            nc.vector.reciprocal(al[:], al[:])
            r2 = rowp.tile([1, 512], f32, tag="r2")
            nc.vector.tensor_tensor(r2[:], mu[:], al[:], ALU.mult)
            ab = rowbp.tile([P, 512], f32, tag="ab")
            nc.gpsimd.partition_broadcast(ab[:], al[:], channels=P)
            rb = rowbp.tile([P, 512], f32, tag="rb")
            nc.gpsimd.partition_broadcast(rb[:], r2[:], channels=P)
            return ab, rb

        def emit_apply(c, src, gcol, bcol, out, ab, rb, dve_js, js=tuple(range(8))):
            """out = (src*alpha - r2)*g + b; split across DVE and Pool."""
            cs = slice(c * 512, (c + 1) * 512)
            for j in js:
                if j in dve_js:
                    sc = scp.tile([P, 512], f32, tag="scv")
                    nc.vector.tensor_tensor(sc[:], src[:, j, cs], ab[:],
                                            ALU.mult)
                    nc.vector.tensor_tensor(sc[:], sc[:], rb[:], ALU.subtract)
                    nc.vector.tensor_scalar(out[:, j, cs], sc[:],
                                            gcol[:, j:j + 1], bcol[:, j:j + 1],
                                            ALU.mult, ALU.add)
                else:
                    sc = scp.tile([P, 512], f32, tag="scp")
                    nc.gpsimd.tensor_tensor(sc[:], src[:, j, cs], ab[:],
                                            ALU.mult)
                    nc.gpsimd.tensor_tensor(sc[:], sc[:], rb[:], ALU.subtract)
                    nc.gpsimd.tensor_scalar(out[:, j, cs], sc[:],
                                            gcol[:, j:j + 1], bcol[:, j:j + 1],
                                            ALU.mult, ALU.add)

        DVE_JS = (0, 1, 2)   # Pool is faster per op; give it the bigger share

        # ---- phase A: QKV + attention, interleaved ---------------------------
        emit_qk(0, pre=(wj0q, wj0k))
        # small consts stream in behind the first weight loads
        nc.sync.dma_start(onpp[:], onespp[:])
        nc.sync.dma_start(on1b[:], ones1b[:])
        nc.sync.dma_start(vb_row[:], vb[None, :])
        # softmax-denominator ones column, value 32 (cancels V's x32)
        vp_col = Vp[:].rearrange("p i (hh e) -> p (i hh) e", e=DH + 1)[:, :, DH]
        nc.scalar.activation(vp_col, onpp[:].to_broadcast((P, 8 * H)), AF.Copy,
                             scale=WSC)
        emit_qk(1, pre=(wj1q, None))
        nc.sync.dma_start(onc[:], onescol[:])
        nc.vector.memset(eps_t[:], EPS)
        nc.sync.dma_start(gb1[:], lng[:].rearrange("(j p) -> p j", p=P))
        nc.sync.dma_start(bb1[:], lnb[:].rearrange("(j p) -> p j", p=P))
        nc.sync.dma_start(gb2[:], fflng[:].rearrange("(j p) -> p j", p=P))
        nc.sync.dma_start(bb2[:], fflnb[:].rearrange("(j p) -> p j", p=P))
        nc.sync.dma_start(f1b_sb[:], f1b[:].rearrange("(j p) -> p j", p=P))
        nc.sync.dma_start(f2b_sb[:], f2b[:].rearrange("(j p) -> p j", p=P))
        nc.sync.dma_start(pb_row[:], pb[None, :])
        # bf16 XT (residual path only) streams behind the fp8 critical path
        for k in range(8):
            nc.sync.dma_start(XT[:, k, :], xTr[:, k, :])
        emit_v(0)
        emit_qk(2)
        # ---- c0 sweep: attention on queries 0..511 for all pairs, QKV
        # projections for later blocks interleaved behind the exp stream.
        emit_attn(0, (0,))
        emit_qk(3)
        emit_resid(CT, XT, 0, slice(0, 512))
        emit_sq(0, CT, (0,), nc.gpsimd)
        emit_v(1)
        emit_attn(1, (0,))
        emit_qk(4)
        emit_resid(CT, XT, 1, slice(0, 512))
        emit_sq(0, CT, (1,), nc.gpsimd)
        emit_attn(2, (0,))
        emit_qk(5)
        emit_resid(CT, XT, 2, slice(0, 512))
        emit_sq(0, CT, (2,), nc.gpsimd)
        emit_attn(3, (0,))
        emit_qk(6)
        emit_resid(CT, XT, 3, slice(0, 512))
        emit_sq(0, CT, (3,), nc.gpsimd)
        emit_attn(4, (0,))
        emit_qk(7)
        emit_resid(CT, XT, 4, slice(0, 512))
        emit_sq(0, CT, (4,), nc.gpsimd)
        emit_attn(5, (0,))
        emit_resid(CT, XT, 5, slice(0, 512))
        emit_sq(0, CT, (5,), nc.gpsimd)
        emit_attn(6, (0,))
        emit_resid(CT, XT, 6, slice(0, 512))
        emit_sq(0, CT, (6,), nc.gpsimd)
        emit_attn(7, (0,))
        emit_resid(CT, XT, 7, slice(0, 512))
        emit_sq(0, CT, (7,), nc.gpsimd)
        # QK/V psum no longer needed; swap those banks to the LN1-c0 stats
        es_qkps.close()
        es_lnA = ExitStack()
        ln_psA = es_lnA.enter_context(
            tc.tile_pool(name="ln_psA", bufs=1, space="PSUM"))
        psS0, psQ0 = emit_stats(ln_psA, 0, CT, "ln1")
        ab0, rb0 = emit_finalize(psS0, psQ0)
        es_lnA.close()
        # apply-c0 split DVE/Pool: the first FF1 filler block gates on it
        emit_apply(0, CT, gb1, bb1, O1T, ab0, rb0, (0, 1, 2))

        # ---- overlap window: attention c1 (exp-bound on ACT) carries the
        # FF1-c0 matmuls as PE filler, one m-block per t-pair.
        es_x8.close()   # free XT8 (projections done)
        es_ffa = ExitStack()
        ffap = es_ffa.enter_context(tc.tile_pool(name="ffap", bufs=1))
        wf1p = es_ffa.enter_context(tc.tile_pool(name="wf1p", bufs=3))
        ff_ps = es_ffa.enter_context(
            tc.tile_pool(name="ff_ps", bufs=2, space="PSUM"))
        H1 = ffap.tile([P, 32, 512], bf16, tag="h1")
        FFT = mid.tile([P, 8, S], f32r, tag="ctff")  # reuse CT buffer

        def emit_ff1_block(m, c, relu_on_act):
            cs = slice(c * 512, (c + 1) * 512)
            wm = wf1p.tile([P, 8, P], bf16, tag="wf1")
            nc.sync.dma_start(wm[:], wf1[m])
            pt = ff_ps.tile([P, 512], f32, tag="ff")
            for k in range(8):
                mm(pt[:], wm[:, k, :], O1T[:, k, cs],
                   start=(k == 0), stop=(k == 7))
            if relu_on_act:
                nc.scalar.activation(H1[:, m, :], pt[:], AF.Relu,
                                     bias=f1b_sb[:, m:m + 1])
            else:
                # DVE relu evac: keeps the overlap window's ACT exp-only
                nc.vector.tensor_scalar(H1[:, m, :], pt[:],
                                        f1b_sb[:, m:m + 1], 0.0,
                                        ALU.add, ALU.max)

        ff1_m = iter(range(32))

        def ff1_filler():
            m = next(ff1_m, None)
            if m is not None:
                emit_ff1_block(m, 0, relu_on_act=False)

        for j in range(8):
            emit_attn(j, (1,), filler=ff1_filler)
            emit_resid(CT, XT, j, slice(512, 1024))
            emit_sq(1, CT, (j,), nc.gpsimd)
        for m in ff1_m:  # any filler slots the attention loop didn't consume
            emit_ff1_block(m, 0, relu_on_act=False)

        es_ph1.close()
        es_qkv.close()   # free XT / XT8 / QT / KT / Vp

        es_ph2 = ExitStack()
        ln_ps1 = es_ph2.enter_context(
            tc.tile_pool(name="ln_ps1", bufs=1, space="PSUM"))

        # LN1-c1 chain; the FF2-c0 matmuls right after keep the PE busy
        # while finalize/apply run on ACT/DVE/Pool.
        psS1, psQ1 = emit_stats(ln_ps1, 1, CT, "ln1")
        ab1, rb1 = emit_finalize(psS1, psQ1)
        emit_apply(1, CT, gb1, bb1, O1T, ab1, rb1, (0, 1, 2))

        # ---- phase C pools (FF + LN2 + proj) ---------------------------------
        es_ph3 = ExitStack()
        ffp = es_ph3.enter_context(tc.tile_pool(name="ffp", bufs=1))
        wf2p = es_ph3.enter_context(tc.tile_pool(name="wf2p", bufs=2))
        ytp = es_ph3.enter_context(tc.tile_pool(name="ytp", bufs=3))
        pj_ps = es_ph3.enter_context(
            tc.tile_pool(name="pj_ps", bufs=2, space="PSUM"))

        O2T = ffp.tile([P, 8, S], bf16, tag="o2t")
        WP = ffp.tile([P, 8, D], bf16, tag="wp")

        def emit_ff1(c):
            for m in range(32):
                emit_ff1_block(m, c, relu_on_act=True)

        def emit_ff2(c, pre=()):
            cs = slice(c * 512, (c + 1) * 512)
            for j in range(8):
                if j < len(pre):
                    w2j = pre[j]
                else:
                    w2j = wf2p.tile([P, 32, P], bf16, tag="w2j")
                    nc.sync.dma_start(w2j[:], wf2[j])
                pt = ff_ps.tile([P, 512], f32, tag="ff")
                for m in range(32):
                    mm(pt[:], w2j[:, m, :], H1[:, m, :],
                       start=(m == 0), stop=(m == 31))
                # fused evac: FFT = (psum + f2b) + O1T  (bias + residual)
                nc.vector.scalar_tensor_tensor(
                    FFT[:, j, cs], pt[:], f2b_sb[:, j:j + 1],
                    O1T[:, j, cs], ALU.add, ALU.add)

        def emit_proj(iset, split_last=False):
            for i in iset:
                yt = ytp.tile([P, D], f32, tag="yt")
                pp = pj_ps.tile([P, D], f32, tag="pj")
                split = split_last and i == iset[-1]
                for dh in range(2):
                    ds_ = slice(dh * 512, (dh + 1) * 512)
                    for k in range(8):
                        mm(pp[:, ds_], O2T[:, k, i * 128:(i + 1) * 128],
                           WP[:, k, ds_], start=(k == 0), stop=False)
                    mm(pp[:, ds_], on1b[:], pb_row[:, ds_],
                       start=False, stop=True)
                    if split:
                        nc.scalar.activation(yt[:, ds_], pp[:, ds_], AF.Copy)
                        nc.sync.dma_start(y[i * 128:(i + 1) * 128, ds_],
                                          yt[:, ds_])
                if not split:
                    nc.scalar.activation(yt[:], pp[:], AF.Copy)
                    nc.sync.dma_start(y[i * 128:(i + 1) * 128, :], yt[:])

        nc.sync.dma_start(WP[:], wp[:])
        emit_ff2(0)
        emit_ff1(1)
        # LN2 c0: chain overlaps FF1 c1 matmuls (residual fused into FF2 evac)
        emit_sq(0, FFT, tuple(range(8)), nc.gpsimd)
        psS2, psQ2 = emit_stats(ln_ps1, 0, FFT, "ln2")
        ab2, rb2 = emit_finalize(psS2, psQ2)
        emit_apply(0, FFT, gb2, bb2, O2T, ab2, rb2, (0, 1))
        emit_ff2(1)
        # LN2 c1 chain overlaps proj i0-i1 (residual fused into FF2 evac)
        emit_sq(1, FFT, tuple(range(8)), nc.gpsimd)
        emit_proj((0, 1))
        psS3, psQ3 = emit_stats(ln_ps1, 1, FFT, "ln2")
        ab3, rb3 = emit_finalize(psS3, psQ3)
        emit_proj((2, 3))
        emit_apply(1, FFT, gb2, bb2, O2T, ab3, rb3, DVE_JS)
        emit_proj((4, 5))
        emit_proj((6,), split_last=True)
        emit_proj((7,), split_last=True)

        es_ph3.close()
        es_ffa.close()
        es_ph2.close()
        es_top.close()

    nc.compile()
    return nc


# ---------------------------------------------------------------- host wrapper
class _SpmdRunner:
    """Compile once, run repeatedly (mirrors bass2jax.run_bass_via_pjrt)."""

    def __init__(self, nc, n_cores):
        import jax
        from jax.sharding import Mesh, PartitionSpec
        from jax.experimental.shard_map import shard_map
        import concourse.mybir as mybir
        from concourse import bass2jax
        from concourse.bass2jax import _bass_exec_p, install_neuronx_cc_hook

        install_neuronx_cc_hook()
        self.n_cores = n_cores
        partition_name = (
            nc.partition_id_tensor.name if nc.partition_id_tensor else None
        )
        in_names, out_names, out_avals, zero_outs = [], [], [], []
        for alloc in nc.m.functions[0].allocations:
            if not isinstance(alloc, mybir.MemoryLocationSet):
                continue
            name = alloc.memorylocations[0].name
            if alloc.kind == "ExternalInput":
                if name != partition_name:
                    in_names.append(name)
            elif alloc.kind == "ExternalOutput":
                shape = tuple(alloc.tensor_shape)
                dtype = mybir.dt.np(alloc.dtype)
                out_names.append(name)
                out_avals.append(jax.core.ShapedArray(shape, dtype))
                zero_outs.append(np.zeros(shape, dtype))
        self.in_names = in_names
        self.out_names = out_names
        self.out_avals = out_avals
        self.zero_outs = zero_outs
        n_params = len(in_names)
        n_outs = len(out_avals)
        all_in_names = in_names + out_names
        if partition_name is not None:
            all_in_names.append(partition_name)
        donate = tuple(range(n_params, n_params + n_outs))

        def _body(*args):
            operands = list(args)
            if partition_name is not None:
                operands.append(bass2jax.partition_id_tensor())
            outs = _bass_exec_p.bind(
                *operands,
                out_avals=tuple(out_avals),
                in_names=tuple(all_in_names),
                out_names=tuple(out_names),
                lowering_input_output_aliases=(),
                sim_require_finite=True,
                sim_require_nnan=True,
                nc=nc,
            )
            return tuple(outs)

        import jax as _jax
        devices = _jax.devices()[:n_cores]
        assert len(devices) == n_cores
        mesh = Mesh(np.asarray(devices), ("core",))
        in_specs = (PartitionSpec("core"),) * (n_params + n_outs)
        out_specs = (PartitionSpec("core"),) * n_outs
        self.fn = _jax.jit(
            shard_map(_body, mesh=mesh, in_specs=in_specs,
                      out_specs=out_specs, check_rep=False),
            donate_argnums=donate,
            keep_unused=True,
        )

    def prep_inputs(self, in_maps):
        per_core = [[np.asarray(m[n]) for n in self.in_names] for m in in_maps]
        return [
            np.concatenate([per_core[c][i] for c in range(self.n_cores)], axis=0)
            for i in range(len(self.in_names))
        ]

    def zeros(self):
        return [
            np.zeros((self.n_cores * z.shape[0], *z.shape[1:]), z.dtype)
            for z in self.zero_outs
        ]

    def run_device(self, concat_in):
        return self.fn(*concat_in, *self.zeros())

    def split(self, out_arrs):
        return [
            {
                name: np.asarray(out_arrs[i]).reshape(
                    self.n_cores, *self.out_avals[i].shape)[c]
                for i, name in enumerate(self.out_names)
            }
            for c in range(self.n_cores)
        ]


def make_in_maps(**inputs):
    import ml_dtypes
    BF16 = np.dtype(ml_dtypes.bfloat16)
    F8 = np.dtype(ml_dtypes.float8_e4m3)
    f32 = np.float32
    q = np.ascontiguousarray(np.asarray(inputs["queries"], dtype=f32))

    def arr(name):
        return np.ascontiguousarray(np.asarray(inputs[name], dtype=f32))

    Qw, Kw, Vw = arr("Qw"), arr("Kw"), arr("Vw")
    proj_w, ff1_w, ff2_w = arr("proj_w"), arr("ff1_w"), arr("ff2_w")

    # packed weight layouts (all-contiguous device DMAs)
    def pack_lhsT(w, nj):  # [dout, din] -> [j, p(k), ko, mc]
        return np.ascontiguousarray(
            w.reshape(nj, P, 8, P).transpose(0, 3, 2, 1))

    def pack_rhs(w):  # [dout, din] -> W^T as [p(k), ko, dout]
        return np.ascontiguousarray(
            w.T.reshape(8, P, w.shape[0]).transpose(1, 0, 2))

    shared = {
        "wq": pack_lhsT(Qw * WSC, 8).astype(F8),
        "wk": pack_lhsT(Kw * WSC, 8).astype(F8),
        "wv": np.ascontiguousarray(
            pack_rhs(Vw * WSC).reshape(P, 8, 2, 512).transpose(2, 0, 1, 3)
        ).astype(F8),
        "wp": pack_rhs(proj_w).astype(BF16),
        "wf1": pack_lhsT(ff1_w, 32).astype(BF16),
        "wf2": np.ascontiguousarray(
            ff2_w.T.reshape(32, P, 8, P).transpose(2, 1, 0, 3)).astype(BF16),
        "qb": arr("Qb") * WSC, "kb": arr("Kb") * WSC,
        "vb": (arr("Vb") * WSC).astype(BF16),
        "f1b": arr("ff1_b"), "f2b": arr("ff2_b"),
        "pb": arr("proj_b").astype(BF16),
        "lng": arr("ln_g"), "lnb": arr("ln_b"),
        "fflng": arr("ffln_g"), "fflnb": arr("ffln_b"),
        "ones1b": np.ones((1, P), dtype=f32).astype(BF16),
        "onescol": np.ones((P, 1), dtype=f32),
        "onespp": np.ones((P, 1), dtype=f32),
    }
    in_maps = []
    for b in range(B):
        m = dict(shared)
        xb = np.ascontiguousarray(
            q[b].T.reshape(8, P, S).transpose(1, 0, 2))
        m["xT"] = xb.astype(BF16)
        m["xT8"] = xb.astype(F8)
        in_maps.append(m)
    return in_maps


def get_runner():
    global _RUNNER
    if _RUNNER is None:
        nc = build_nc()
        _RUNNER = _SpmdRunner(nc, NCORES)
    return _RUNNER


def kernel(**inputs):
    runner = get_runner()
    in_maps = make_in_maps(**inputs)
    res = runner.split(runner.run_device(runner.prep_inputs(in_maps)))
    out = np.stack([res[c]["y"] for c in range(NCORES)], axis=0)
    return out.astype(np.float32)
